# revision 1
# baseline (speedup 1.0000x reference)
"""F2NetHead Trainium2 kernel (8 NeuronCores, Bass/Tile).

Reference computation (per batch b):
    qog = x @ W_qog.T + b_qog ; Q,O,G = split(qog)
    cq  = silu(conv1d(Q, conv_w, pad=1) + conv_b)          # mixes channels
    l   = (cq @ w_a.T) / sqrt(d)
    attn= softmax(l, axis=seq)
    glob= sum_seq(Q * attn)                                 # [1, d]
    P   = O * glob
    L   = silu(G) * cumsum(P, axis=seq)
    R   = L @ W_out.T + b_out

Sharding: 8 cores = 4 batches x 2 sequence halves. Each core computes
2048 tokens of one batch. The host supplies the x-rows with a 1-token
halo on each side (zero rows at the sequence edges) so the conv needs no
neighbor exchange. The only cross-core communication is a pairwise
AllReduce of 3 small [d] vectors per batch:
    E  = sum_seq exp(l)            (softmax denominator)
    N  = sum_seq Q * exp(l)        (softmax numerator of glob)
    sx = sum of this half's x rows (first half only)
The cumsum offset of the second half is glob * (W_O @ sx_h0 + T*b_O),
i.e. the first half's P-column-sums, computed without materializing O.

On-chip layout is feature-major ([d partitions, tokens free]) so every
sequence-axis op (softmax sums, global sum, cumsum) is a free-dim op.
All matmuls run in float32r (full PE rate, ~1.6e-4 rel err on hw);
silu is computed as x*sigmoid(x) (ACT sigmoid + DVE multiply).
"""

import numpy as np

import concourse.bacc as bacc
import concourse.mybir as mybir
import concourse.tile as tile
from concourse.bass_utils import run_bass_kernel_spmd

F32 = mybir.dt.float32
F32R = mybir.dt.float32r
AF = mybir.ActivationFunctionType
OP = mybir.AluOpType

B, S, D, DM = 4, 4096, 1024, 1024
N_CORES = 8
T = S // 2            # tokens per core
TH = T + 2            # with halo
DT = D // 128         # d tiles (8)
KT = DM // 128        # contraction tiles (8)
ABLK = 410            # phase A token block (5 blocks over TH=2050)
BBLK = 512            # phase B token block (4 blocks over T)
CBLK = 256            # phase C token block (8 blocks over T)
SCALE = 1.0 / float(np.sqrt(D))


def _emit(tc, nc, prm, phases=5):
    reps = 1
    if phases >= 100:
        reps, phases = phases // 100, 5
    for _ in range(reps):
        _emit_once(tc, nc, prm, phases)


def _emit_once(tc, nc, prm, phases):
    x, wqt, wct, wat, wot = prm["x"], prm["wqt"], prm["wct"], prm["wat"], prm["wot"]
    bq, bo, bg, cb, bout = prm["bq"], prm["bo"], prm["bg"], prm["cb"], prm["bout"]
    hf0, hf1, r_out = prm["hf0"], prm["hf1"], prm["r"]

    with (
        tc.tile_pool(name="cols", bufs=1) as cols,
        tc.tile_pool(name="woo", bufs=1) as woo_pool,
        tc.tile_pool(name="dram", bufs=1, space="DRAM") as dram,
    ):
        # W_O^T loaded up-front (fits alongside every phase) so phase C's
        # O-matmuls and the offset matvec start right after the allreduce
        # instead of stalling on an 8 MiB weight load
        woo = woo_pool.tile([128, KT, DT, 128], F32R)
        for kc in range(KT):
            nc.sync.dma_start(
                woo[:, kc, :, :],
                wqt[kc * 128:(kc + 1) * 128, D:2 * D]
                .rearrange("p (a m) -> p a m", m=128).bitcast(F32R),
            )
        # per-partition bias / flag columns ([128, DT] with d = a*128 + p)
        bq_sb = cols.tile([128, DT], F32)
        bo_sb = cols.tile([128, DT], F32)
        bg_sb = cols.tile([128, DT], F32)
        cb_sb = cols.tile([128, DT], F32)
        bout_sb = cols.tile([128, DT], F32)
        hf0_sb = cols.tile([128, 1], F32)
        hf1_sb = cols.tile([128, 1], F32)
        for t_, d_ in ((bq_sb, bq), (bo_sb, bo), (bg_sb, bg), (cb_sb, cb),
                       (bout_sb, bout), (hf0_sb, hf0), (hf1_sb, hf1)):
            nc.sync.dma_start(t_[:], d_[:])

        # accumulators that survive across phases
        sx_cols = cols.tile([128, KT, 5], F32)      # per-A-block x sums
        e_cols = cols.tile([128, DT * 4], F32)      # per-(a,B-block) exp sums
        n_cols = cols.tile([128, DT * 4], F32)      # per-(a,B-block) Q*exp sums
        stage = cols.tile([128, 3 * DT], F32)       # allreduce staging
        red = cols.tile([128, 3 * DT], F32)         # allreduce result
        glob = cols.tile([128, DT], F32)
        offset = cols.tile([128, DT], F32)

        # ---------------- phase A: Q^T over TH halo'd tokens ----------------
        with tc.tile_pool(name="qt", bufs=1) as qt_pool:
            qt = qt_pool.tile([128, DT, TH], F32R)
            with (
                tc.tile_pool(name="wq", bufs=1) as wq_pool,
                tc.tile_pool(name="xa", bufs=2) as xa_pool,
                tc.tile_pool(name="psa", bufs=8, space="PSUM") as psa,
            ):
                wq = [wq_pool.tile([128, DT, 128], F32R, tag=f"wq{kc}",
                                   name=f"wq{kc}") for kc in range(KT)]
                for kc in range(KT):
                    nc.sync.dma_start(
                        wq[kc][:],
                        wqt[kc * 128:(kc + 1) * 128, 0:D]
                        .rearrange("p (a m) -> p a m", m=128).bitcast(F32R),
                    )
                for blk in range(5):
                    t0 = blk * ABLK
                    xt = [xa_pool.tile([128, ABLK], F32R, tag=f"xa{kc}",
                                       name=f"xa{kc}") for kc in range(KT)]
                    for kc in range(KT):
                        nc.sync.dma_start(
                            xt[kc][:],
                            x[kc * 128:(kc + 1) * 128, t0:t0 + ABLK].bitcast(F32R),
                        )
                    # x column-sums over main (non-halo) tokens for cumsum offset
                    lo = 1 - t0 if t0 < 1 else 0
                    hi = ABLK - max(0, t0 + ABLK - (TH - 1))
                    for kc in range(KT):
                        nc.vector.tensor_reduce(
                            sx_cols[:, kc, blk:blk + 1], xt[kc][:, lo:hi],
                            axis=mybir.AxisListType.X, op=OP.add,
                        )
                    for a in range(DT):
                        ps = psa.tile([128, ABLK], F32, tag="ps")
                        for kc in range(KT):
                            nc.tensor.matmul(
                                ps[:], wq[kc][:, a, :], xt[kc][:],
                                start=(kc == 0), stop=(kc == KT - 1),
                            )
                        nc.vector.tensor_scalar_add(
                            qt[:, a, t0:t0 + ABLK], ps[:], bq_sb[:, a:a + 1]
                        )

            if phases == 1:
                for a in range(DT):
                    nc.sync.dma_start(
                        r_out[a * 128:(a + 1) * 128, 0:T],
                        qt[:, a, 1:T + 1].bitcast(F32),
                    )
                return

            # ------------- phase B1: cq^T = silu(conv(Q)) -------------
            with tc.tile_pool(name="cq", bufs=1) as cq_pool:
                cq = cq_pool.tile([128, DT, T], F32R)
                with (
                    tc.tile_pool(name="wc", bufs=2) as wc_pool,
                    tc.tile_pool(name="psb", bufs=8, space="PSUM") as psb,
                ):
                    for a in range(DT):
                        wc = wc_pool.tile([128, 3, KT, 128], F32R, tag="wc")
                        for k3 in range(3):
                            nc.sync.dma_start(
                                wc[:, k3, :, :],
                                wct[k3, :, a * 128:(a + 1) * 128]
                                .rearrange("(kc p) m -> p kc m", p=128)
                                .bitcast(F32R),
                            )
                        for blk in range(T // BBLK):
                            t0 = blk * BBLK
                            ps = psb.tile([128, BBLK], F32, tag="ps")
                            first = True
                            for k3 in range(3):
                                for kc in range(KT):
                                    nc.tensor.matmul(
                                        ps[:], wc[:, k3, kc, :],
                                        qt[:, kc, t0 + k3:t0 + k3 + BBLK],
                                        start=first,
                                        stop=(k3 == 2 and kc == KT - 1),
                                    )
                                    first = False
                            sig = wc_pool.tile([128, BBLK], F32, tag="sig")
                            nc.scalar.activation(
                                sig[:], ps[:], AF.Sigmoid, bias=cb_sb[:, a:a + 1]
                            )
                            nc.vector.scalar_tensor_tensor(
                                cq[:, a, t0:t0 + BBLK], ps[:], cb_sb[:, a:a + 1],
                                sig[:], OP.add, OP.mult,
                            )

                if phases == 2:
                    for a in range(DT):
                        nc.sync.dma_start(
                            r_out[a * 128:(a + 1) * 128, 0:T],
                            cq[:, a, :].bitcast(F32),
                        )
                    return

                # ------- phase B2: E/N partial sums from exp(logits) -------
                with (
                    tc.tile_pool(name="wa", bufs=1) as wa_pool,
                    tc.tile_pool(name="ex", bufs=2) as ex_pool,
                    tc.tile_pool(name="psl", bufs=8, space="PSUM") as psl,
                ):
                    wa = [wa_pool.tile([128, DT, 128], F32R, tag=f"wa{kc}",
                                       name=f"wa{kc}") for kc in range(KT)]
                    for kc in range(KT):
                        nc.sync.dma_start(
                            wa[kc][:],
                            wat[kc * 128:(kc + 1) * 128, :]
                            .rearrange("p (a m) -> p a m", m=128).bitcast(F32R),
                        )
                    for blk in range(T // BBLK):
                        t0 = blk * BBLK
                        for a in range(DT):
                            ps = psl.tile([128, BBLK], F32, tag="ps")
                            for kc in range(KT):
                                nc.tensor.matmul(
                                    ps[:], wa[kc][:, a, :], cq[:, kc, t0:t0 + BBLK],
                                    start=(kc == 0), stop=(kc == KT - 1),
                                )
                            expl = ex_pool.tile([128, BBLK], F32, tag="expl")
                            idx = a * 4 + blk
                            nc.scalar.activation(
                                expl[:], ps[:], AF.Exp, scale=SCALE,
                                accum_out=e_cols[:, idx:idx + 1],
                            )
                            prod = ex_pool.tile([128, BBLK], F32, tag="prod")
                            nc.vector.scalar_tensor_tensor(
                                prod[:], expl[:], 0.0,
                                qt[:, a, t0 + 1:t0 + 1 + BBLK].bitcast(F32),
                                OP.add, OP.mult,
                                accum_out=n_cols[:, idx:idx + 1],
                            )

        if phases == 3:
            nc.sync.dma_start(r_out[0:128, 0:32].rearrange("p t -> p t"), e_cols[:])
            nc.sync.dma_start(r_out[128:256, 0:32], n_cols[:])
            return

        # ---------------- allreduce E, N, sx over the seq pair ----------------
        nc.vector.tensor_reduce(
            stage[:, 0:DT], e_cols[:].rearrange("p (a b) -> p a b", b=4),
            axis=mybir.AxisListType.X, op=OP.add,
        )
        nc.vector.tensor_reduce(
            stage[:, DT:2 * DT], n_cols[:].rearrange("p (a b) -> p a b", b=4),
            axis=mybir.AxisListType.X, op=OP.add,
        )
        # x sums (main tokens) masked to the first half: slot = sx * (1-h)
        nc.vector.tensor_reduce(
            stage[:, 2 * DT:3 * DT], sx_cols[:],
            axis=mybir.AxisListType.X, op=OP.add,
        )
        nc.vector.tensor_scalar_mul(
            stage[:, 2 * DT:3 * DT], stage[:, 2 * DT:3 * DT], hf0_sb[:, 0:1]
        )
        if phases == 99:
            # timing-model variant: skip the collective (TimelineSim
            # cannot model collectives); copy stage -> red locally
            nc.vector.tensor_copy(red[:], stage[:])
        else:
            cc_in = dram.tile([128, 3 * DT], F32)
            cc_out = dram.tile([128, 3 * DT], F32)
            nc.sync.dma_start(cc_in[:], stage[:])
            nc.gpsimd.collective_compute(
                "AllReduce", OP.add,
                replica_groups=[[0, 1], [2, 3], [4, 5], [6, 7]],
                ins=[cc_in.opt()], outs=[cc_out.opt()],
            )
            nc.sync.dma_start(red[:], cc_out[:])

        # glob = N / E
        recip = cols.tile([128, DT], F32)
        nc.vector.reciprocal(recip[:], red[:, 0:DT])
        nc.vector.tensor_mul(glob[:], red[:, DT:2 * DT], recip[:])

        # ---------------- phase C: O,G -> P -> cumsum -> L -> R ----------------
        with (
            tc.tile_pool(name="wog", bufs=1) as wog_pool,
            tc.tile_pool(name="wo2", bufs=1) as wo2_pool,
        ):
            wog = wog_pool.tile([128, KT, DT, 128], F32R)
            for kc in range(KT):
                nc.sync.dma_start(
                    wog[:, kc, :, :],
                    wqt[kc * 128:(kc + 1) * 128, 2 * D:3 * D]
                    .rearrange("p (a m) -> p a m", m=128).bitcast(F32R),
                )
            wo2 = wo2_pool.tile([128, KT, DT, 128], F32R)
            for kc in range(KT):
                nc.sync.dma_start(
                    wo2[:, kc, :, :],
                    wot[kc * 128:(kc + 1) * 128, :]
                    .rearrange("p (a m) -> p a m", m=128).bitcast(F32R),
                )

            # cumsum offset for the second half: glob * (W_O @ sx_h0 + T*b_O)
            # (plain fp32 matmul: fp32r rejects a size-1 moving operand)
            bo_t = cols.tile([128, DT], F32)
            nc.vector.tensor_scalar_mul(bo_t[:], bo_sb[:], float(T))
            offv = cols.tile([128, DT], F32)
            with tc.tile_pool(name="psm", bufs=2, space="PSUM") as psm:
                for a in range(DT):
                    ps = psm.tile([128, 1], F32, tag="ps")
                    for kc in range(KT):
                        nc.tensor.matmul(
                            ps[:], woo[:, kc, a, :].bitcast(F32),
                            red[:, 2 * DT + kc:2 * DT + kc + 1],
                            start=(kc == 0), stop=(kc == KT - 1),
                        )
                    nc.vector.tensor_scalar_add(
                        offv[:, a:a + 1], ps[:], bo_t[:, a:a + 1]
                    )
            nc.vector.tensor_mul(offset[:], offv[:], glob[:])
            nc.vector.tensor_scalar_mul(offset[:], offset[:], hf1_sb[:, 0:1])
            boglob = cols.tile([128, DT], F32)
            nc.vector.tensor_mul(boglob[:], bo_sb[:], glob[:])

            if phases == 4:
                nc.sync.dma_start(r_out[0:128, 0:DT], offset[:])
                nc.sync.dma_start(r_out[128:256, 0:DT], glob[:])
                return

            with (
                tc.tile_pool(name="xc", bufs=2) as xc_pool,
                tc.tile_pool(name="blkb", bufs=2) as blk_pool,
                tc.tile_pool(name="psc", bufs=8, space="PSUM") as psc,
            ):
                c_prev = None
                nblk = T // CBLK
                if 50 <= phases < 99:
                    nblk = phases - 50
                for blk in range(nblk):
                    t0 = blk * CBLK
                    xt = xc_pool.tile([128, KT, CBLK], F32R, tag="xc")
                    for kc in range(KT):
                        nc.sync.dma_start(
                            xt[:, kc, :],
                            x[kc * 128:(kc + 1) * 128, t0 + 1:t0 + 1 + CBLK]
                            .bitcast(F32R),
                        )
                    pt = blk_pool.tile([128, DT, CBLK], F32, tag="pt")
                    ct = blk_pool.tile([128, DT, CBLK], F32, tag="ct")
                    carry = xc_pool.tile([128, DT], F32, tag="carry")
                    gt = blk_pool.tile([128, DT, CBLK], F32, tag="gt")
                    lt = blk_pool.tile([128, DT, CBLK], F32R, tag="lt")
                    rt = blk_pool.tile([128, DT, CBLK], F32, tag="rt")
                    for a in range(DT):
                        ps = psc.tile([128, CBLK], F32, tag="ps")
                        for kc in range(KT):
                            nc.tensor.matmul(
                                ps[:], woo[:, kc, a, :], xt[:, kc, :],
                                start=(kc == 0), stop=(kc == KT - 1),
                            )
                        # P = (O + b_o) * glob = O*glob + (b_o*glob), on ACT
                        nc.scalar.activation(
                            pt[:, a, :], ps[:], AF.Identity,
                            bias=boglob[:, a:a + 1], scale=glob[:, a:a + 1],
                        )
                        init = (offset[:, a:a + 1] if c_prev is None
                                else c_prev[:, a:a + 1])
                        nc.vector.tensor_tensor_scan(
                            ct[:, a, :], pt[:, a, :], pt[:, a, :], init,
                            OP.add, OP.bypass,
                        )
                    # carry the last cumsum column via ACT so the next
                    # block's scan does not read a scan output directly
                    nc.scalar.copy(carry[:], ct[:, :, CBLK - 1:CBLK])
                    for a in range(DT):
                        ps = psc.tile([128, CBLK], F32, tag="ps")
                        for kc in range(KT):
                            nc.tensor.matmul(
                                ps[:], wog[:, kc, a, :], xt[:, kc, :],
                                start=(kc == 0), stop=(kc == KT - 1),
                            )
                        sig = xc_pool.tile([128, CBLK], F32, tag="sig")
                        nc.scalar.activation(
                            sig[:], ps[:], AF.Sigmoid, bias=bg_sb[:, a:a + 1]
                        )
                        nc.vector.scalar_tensor_tensor(
                            gt[:, a, :], ps[:], bg_sb[:, a:a + 1], sig[:],
                            OP.add, OP.mult,
                        )
                        nc.vector.tensor_mul(lt[:, a, :], gt[:, a, :], ct[:, a, :])
                    for a in range(DT):
                        ps = psc.tile([128, CBLK], F32, tag="ps")
                        for kc in range(KT):
                            nc.tensor.matmul(
                                ps[:], wo2[:, kc, a, :], lt[:, kc, :],
                                start=(kc == 0), stop=(kc == KT - 1),
                            )
                        nc.scalar.activation(
                            rt[:, a, :], ps[:], AF.Identity,
                            bias=bout_sb[:, a:a + 1],
                        )
                    for a in range(DT):
                        nc.sync.dma_start(
                            r_out[a * 128:(a + 1) * 128, t0:t0 + CBLK],
                            rt[:, a, :],
                        )
                    c_prev = carry


_CACHE = {}


def _build(phases=5):
    if phases in _CACHE:
        return _CACHE[phases]
    nc = bacc.Bacc(None, target_bir_lowering=False, num_devices=N_CORES)
    prm = {
        "x": nc.declare_dram_parameter("x", [DM, TH], F32, isOutput=False),
        "wqt": nc.declare_dram_parameter("wqt", [DM, 3 * D], F32, isOutput=False),
        "wct": nc.declare_dram_parameter("wct", [3, D, D], F32, isOutput=False),
        "wat": nc.declare_dram_parameter("wat", [D, D], F32, isOutput=False),
        "wot": nc.declare_dram_parameter("wot", [D, D], F32, isOutput=False),
        "bq": nc.declare_dram_parameter("bq", [128, DT], F32, isOutput=False),
        "bo": nc.declare_dram_parameter("bo", [128, DT], F32, isOutput=False),
        "bg": nc.declare_dram_parameter("bg", [128, DT], F32, isOutput=False),
        "cb": nc.declare_dram_parameter("cb", [128, DT], F32, isOutput=False),
        "bout": nc.declare_dram_parameter("bout", [128, DT], F32, isOutput=False),
        "hf0": nc.declare_dram_parameter("hf0", [128, 1], F32, isOutput=False),
        "hf1": nc.declare_dram_parameter("hf1", [128, 1], F32, isOutput=False),
        "r": nc.declare_dram_parameter("r", [DM, T], F32, isOutput=True),
    }
    with tile.TileContext(nc, num_cores=N_CORES) as tc:
        _emit(tc, nc, prm, phases)
    nc.compile()
    _CACHE[phases] = nc
    return nc


def make_in_maps(x, W_qog, b_qog, conv_w, conv_b, w_a, W_out, b_out):
    f = np.float32
    x = np.asarray(x, f)
    wqt = np.ascontiguousarray(np.asarray(W_qog, f).T)          # [dm, 3d]
    wct = np.ascontiguousarray(np.asarray(conv_w, f).transpose(2, 1, 0))
    wat = np.ascontiguousarray(np.asarray(w_a, f).T)
    wot = np.ascontiguousarray(np.asarray(W_out, f).T)

    def col(v):  # [d] -> [128, DT] with d = a*128 + p
        return np.ascontiguousarray(np.asarray(v, f).reshape(DT, 128).T)

    b_qog = np.asarray(b_qog, f)
    bq, bo, bg = col(b_qog[:D]), col(b_qog[D:2 * D]), col(b_qog[2 * D:])
    cb, bout = col(conv_b), col(b_out)

    in_maps = []
    for c in range(N_CORES):
        b, h = c // 2, c % 2
        t0 = h * T
        xs = np.zeros((TH, DM), f)
        xs[1:T + 1] = x[b, t0:t0 + T]
        if t0 > 0:
            xs[0] = x[b, t0 - 1]
        if t0 + T < S:
            xs[T + 1] = x[b, t0 + T]
        xs = np.ascontiguousarray(xs.T)            # [DM, TH] feature-major
        in_maps.append({
            "x": xs, "wqt": wqt, "wct": wct, "wat": wat, "wot": wot,
            "bq": bq, "bo": bo, "bg": bg, "cb": cb, "bout": bout,
            "hf0": np.full((128, 1), 1.0 - h, f),
            "hf1": np.full((128, 1), float(h), f),
        })
    return in_maps


def kernel(x, W_qog, b_qog, conv_w, conv_b, w_a, W_out, b_out):
    nc = _build(5)
    in_maps = make_in_maps(x, W_qog, b_qog, conv_w, conv_b, w_a, W_out, b_out)
    res = None
    for attempt in range(3):
        try:
            res = run_bass_kernel_spmd(nc, in_maps, list(range(N_CORES)))
            break
        except Exception:
            # the execution path through the device bridge is occasionally
            # flaky (worker hangup); reset the backend and retry
            if attempt == 2:
                raise
            import jax

            try:
                jax.clear_backends()
            except Exception:
                pass
            import time

            time.sleep(5)
    out = np.empty((B, S, DM), np.float32)
    for c in range(N_CORES):
        b, h = c // 2, c % 2
        out[b, h * T:(h + 1) * T, :] = res.results[c]["r"].T
    return out



# revision 2
# speedup vs baseline: 10.9199x; 10.9199x over previous
"""F2NetHead Trainium2 kernel (8 NeuronCores, Bass/Tile) — v2.

Reference computation (per batch b):
    qog = x @ W_qog.T + b_qog ; Q,O,G = split(qog)
    cq  = silu(conv1d(Q, conv_w, pad=1) + conv_b)          # mixes channels
    l   = (cq @ w_a.T) / sqrt(d)
    attn= softmax(l, axis=seq)
    glob= sum_seq(Q * attn)                                 # [1, d]
    P   = O * glob
    L   = silu(G) * cumsum(P, axis=seq)
    R   = L @ W_out.T + b_out

Sharding: 8 cores = 4 batches x 2 sequence halves; x arrives with a
1-token halo so the conv needs no neighbor exchange. Per batch pair the
only comm is an AllReduce of 3 [d] vectors: E = sum exp(l),
N = sum Q*exp(l), sx = sum of half-0's x rows.

v2 changes vs v1:
  *  glob factors out of the cumsum:  cumsum(P) = glob * cumsum(O + b_o)
     and the half-1 offset = glob * (W_O @ sx + T b_o).  Phase C's O/G
     matmuls and the raw cumsum therefore have NO dependency on the
     collective; only a per-block tensor_scalar (+offset, *glob) and the
     W_out matmul consume the allreduce result.  SLACK blocks of O/G
     work are emitted before the first allreduce-dependent instruction
     on each engine queue, hiding the collective latency.
  *  weight DMA is staged: W_Q before phase A, conv weights + w_a during
     B1, W_O during B2, W_G/W_out at C start — removes the 8 MiB
     startup stall and the B1->B2 weight-load gap.

On-chip layout is feature-major ([d partitions, tokens free]) so every
sequence-axis op (softmax sums, global sum, cumsum) is a free-dim op.
All matmuls run in float32r (full PE rate); silu is x*sigmoid(x).
"""

import numpy as np

import concourse.bacc as bacc
import concourse.mybir as mybir
import concourse.tile as tile
from concourse.bass_utils import run_bass_kernel_spmd

F32 = mybir.dt.float32
F32R = mybir.dt.float32r
AF = mybir.ActivationFunctionType
OP = mybir.AluOpType

B, S, D, DM = 4, 4096, 1024, 1024
N_CORES = 8
T = S // 2            # tokens per core
TH = T + 2            # with halo
DT = D // 128         # d tiles (8)
KT = DM // 128        # contraction tiles (8)
ABLK = 410            # phase A token block (5 blocks over TH=2050)
BBLK = 512            # phase B token block (4 blocks over T)
CBLK = 256            # phase C token block (8 blocks over T)
SLACK = 2             # C blocks of O/G work emitted ahead of any
                      # allreduce-dependent instruction
SCALE = 1.0 / float(np.sqrt(D))


def _emit(tc, nc, prm, phases=5):
    reps = 1
    if phases >= 100:
        reps, phases = phases // 100, 5
    for _ in range(reps):
        _emit_once(tc, nc, prm, phases)


def _emit_once(tc, nc, prm, phases):
    x, wqt, wct, wat, wot = prm["x"], prm["wqt"], prm["wct"], prm["wat"], prm["wot"]
    bq, bo, bg, cb, bout = prm["bq"], prm["bo"], prm["bg"], prm["cb"], prm["bout"]
    hf0, hf1, r_out = prm["hf0"], prm["hf1"], prm["r"]

    with (
        tc.tile_pool(name="cols", bufs=1) as cols,
        tc.tile_pool(name="woo", bufs=1) as woo_pool,
        tc.tile_pool(name="dram", bufs=1, space="DRAM") as dram,
    ):
        # per-partition bias / flag columns ([128, DT] with d = a*128 + p)
        bq_sb = cols.tile([128, DT], F32)
        bo_sb = cols.tile([128, DT], F32)
        bg_sb = cols.tile([128, DT], F32)
        cb_sb = cols.tile([128, DT], F32)
        bout_sb = cols.tile([128, DT], F32)
        hf0_sb = cols.tile([128, 1], F32)
        hf1_sb = cols.tile([128, 1], F32)
        for t_, d_ in ((bq_sb, bq), (bo_sb, bo), (bg_sb, bg), (cb_sb, cb),
                       (bout_sb, bout), (hf0_sb, hf0), (hf1_sb, hf1)):
            nc.sync.dma_start(t_[:], d_[:])

        # accumulators that survive across phases
        sx_cols = cols.tile([128, KT, 5], F32)      # per-A-block x sums
        e_cols = cols.tile([128, DT * 4], F32)      # per-(a,B-block) exp sums
        n_cols = cols.tile([128, DT * 4], F32)      # per-(a,B-block) Q*exp sums
        stage = cols.tile([128, 3 * DT], F32)       # allreduce staging
        red = cols.tile([128, 3 * DT], F32)         # allreduce result
        glob = cols.tile([128, DT], F32)
        offh = cols.tile([128, DT], F32)            # hf1*(W_O@sx + T b_o)
        zcol = cols.tile([128, 1], F32)             # scan init for block 0
        nc.vector.memset(zcol[:], 0.0)

        # ---------------- phase A: Q^T over TH halo'd tokens ----------------
        with tc.tile_pool(name="qt", bufs=1) as qt_pool:
            qt = qt_pool.tile([128, DT, TH], F32R)
            with (
                tc.tile_pool(name="wq", bufs=1) as wq_pool,
                tc.tile_pool(name="xa", bufs=2) as xa_pool,
                tc.tile_pool(name="psa", bufs=8, space="PSUM") as psa,
            ):
                wq = [wq_pool.tile([128, DT, 128], F32R, tag=f"wq{kc}",
                                   name=f"wq{kc}") for kc in range(KT)]
                # block-0 x tiles interleave with the wq loads so the first
                # contraction chain can start as soon as pair 0 lands
                xt0 = [xa_pool.tile([128, ABLK], F32R, tag=f"xa{kc}",
                                    name=f"xa{kc}") for kc in range(KT)]
                for kc in range(KT):
                    nc.sync.dma_start(
                        wq[kc][:],
                        wqt[kc * 128:(kc + 1) * 128, 0:D]
                        .rearrange("p (a m) -> p a m", m=128).bitcast(F32R),
                    )
                    nc.sync.dma_start(
                        xt0[kc][:],
                        x[kc * 128:(kc + 1) * 128, 0:ABLK].bitcast(F32R),
                    )
                for blk in range(5):
                    t0 = blk * ABLK
                    if blk == 0:
                        xt = xt0
                    else:
                        xt = [xa_pool.tile([128, ABLK], F32R, tag=f"xa{kc}",
                                           name=f"xa{kc}") for kc in range(KT)]
                        for kc in range(KT):
                            nc.sync.dma_start(
                                xt[kc][:],
                                x[kc * 128:(kc + 1) * 128, t0:t0 + ABLK]
                                .bitcast(F32R),
                            )
                    # x column-sums over main (non-halo) tokens for cumsum offset
                    lo = 1 - t0 if t0 < 1 else 0
                    hi = ABLK - max(0, t0 + ABLK - (TH - 1))
                    for kc in range(KT):
                        nc.vector.tensor_reduce(
                            sx_cols[:, kc, blk:blk + 1], xt[kc][:, lo:hi],
                            axis=mybir.AxisListType.X, op=OP.add,
                        )
                    for a in range(DT):
                        ps = psa.tile([128, ABLK], F32, tag="ps")
                        for kc in range(KT):
                            nc.tensor.matmul(
                                ps[:], wq[kc][:, a, :], xt[kc][:],
                                start=(kc == 0), stop=(kc == KT - 1),
                            )
                        nc.vector.tensor_scalar_add(
                            qt[:, a, t0:t0 + ABLK], ps[:], bq_sb[:, a:a + 1]
                        )

            if phases == 1:
                for a in range(DT):
                    nc.sync.dma_start(
                        r_out[a * 128:(a + 1) * 128, 0:T],
                        qt[:, a, 1:T + 1].bitcast(F32),
                    )
                return

            # ------------- phase B1: cq^T = silu(conv(Q)) -------------
            with tc.tile_pool(name="cq", bufs=1) as cq_pool:
                cq = cq_pool.tile([128, DT, T], F32R)
                with (
                    tc.tile_pool(name="wc", bufs=2) as wc_pool,
                    tc.tile_pool(name="psb", bufs=8, space="PSUM") as psb,
                ):
                    for a in range(DT):
                        wc = wc_pool.tile([128, 3, KT, 128], F32R, tag="wc")
                        for k3 in range(3):
                            nc.sync.dma_start(
                                wc[:, k3, :, :],
                                wct[k3, :, a * 128:(a + 1) * 128]
                                .rearrange("(kc p) m -> p kc m", p=128)
                                .bitcast(F32R),
                            )
                        for blk in range(T // BBLK):
                            t0 = blk * BBLK
                            ps = psb.tile([128, BBLK], F32, tag="ps")
                            first = True
                            for k3 in range(3):
                                for kc in range(KT):
                                    nc.tensor.matmul(
                                        ps[:], wc[:, k3, kc, :],
                                        qt[:, kc, t0 + k3:t0 + k3 + BBLK],
                                        start=first,
                                        stop=(k3 == 2 and kc == KT - 1),
                                    )
                                    first = False
                            sig = wc_pool.tile([128, BBLK], F32, tag="sig")
                            nc.scalar.activation(
                                sig[:], ps[:], AF.Sigmoid, bias=cb_sb[:, a:a + 1]
                            )
                            nc.vector.scalar_tensor_tensor(
                                cq[:, a, t0:t0 + BBLK], ps[:], cb_sb[:, a:a + 1],
                                sig[:], OP.add, OP.mult,
                            )

                if phases == 2:
                    for a in range(DT):
                        nc.sync.dma_start(
                            r_out[a * 128:(a + 1) * 128, 0:T],
                            cq[:, a, :].bitcast(F32),
                        )
                    return

                # ------- phase B2: E/N partial sums from exp(logits) -------
                # w_a loads first (needed immediately); W_O^T queued behind
                # it, streaming during B2 compute (offset matvec + phase C)
                with (
                    tc.tile_pool(name="wa", bufs=1) as wa_pool,
                    tc.tile_pool(name="ex", bufs=2) as ex_pool,
                    tc.tile_pool(name="psl", bufs=8, space="PSUM") as psl,
                ):
                    wa = [wa_pool.tile([128, DT, 128], F32R, tag=f"wa{kc}",
                                       name=f"wa{kc}") for kc in range(KT)]
                    for kc in range(KT):
                        nc.sync.dma_start(
                            wa[kc][:],
                            wat[kc * 128:(kc + 1) * 128, :]
                            .rearrange("p (a m) -> p a m", m=128).bitcast(F32R),
                        )
                    woo = woo_pool.tile([128, KT, DT, 128], F32R)
                    for kc in range(KT):
                        nc.sync.dma_start(
                            woo[:, kc, :, :],
                            wqt[kc * 128:(kc + 1) * 128, D:2 * D]
                            .rearrange("p (a m) -> p a m", m=128).bitcast(F32R),
                        )
                    for blk in range(T // BBLK):
                        t0 = blk * BBLK
                        for a in range(DT):
                            ps = psl.tile([128, BBLK], F32, tag="ps")
                            for kc in range(KT):
                                nc.tensor.matmul(
                                    ps[:], wa[kc][:, a, :],
                                    cq[:, kc, t0:t0 + BBLK],
                                    start=(kc == 0), stop=(kc == KT - 1),
                                )
                            expl = ex_pool.tile([128, BBLK], F32, tag="expl")
                            idx = a * 4 + blk
                            nc.scalar.activation(
                                expl[:], ps[:], AF.Exp, scale=SCALE,
                                accum_out=e_cols[:, idx:idx + 1],
                            )
                            prod = ex_pool.tile([128, BBLK], F32, tag="prod")
                            nc.vector.scalar_tensor_tensor(
                                prod[:], expl[:], 0.0,
                                qt[:, a, t0 + 1:t0 + 1 + BBLK].bitcast(F32),
                                OP.add, OP.mult,
                                accum_out=n_cols[:, idx:idx + 1],
                            )

        if phases == 3:
            nc.sync.dma_start(r_out[0:128, 0:32].rearrange("p t -> p t"), e_cols[:])
            nc.sync.dma_start(r_out[128:256, 0:32], n_cols[:])
            return

        # ---------------- allreduce E, N, sx over the seq pair ----------------
        nc.vector.tensor_reduce(
            stage[:, 0:DT], e_cols[:].rearrange("p (a b) -> p a b", b=4),
            axis=mybir.AxisListType.X, op=OP.add,
        )
        nc.vector.tensor_reduce(
            stage[:, DT:2 * DT], n_cols[:].rearrange("p (a b) -> p a b", b=4),
            axis=mybir.AxisListType.X, op=OP.add,
        )
        # x sums (main tokens) masked to the first half: slot = sx * (1-h)
        nc.vector.tensor_reduce(
            stage[:, 2 * DT:3 * DT], sx_cols[:],
            axis=mybir.AxisListType.X, op=OP.add,
        )
        nc.vector.tensor_scalar_mul(
            stage[:, 2 * DT:3 * DT], stage[:, 2 * DT:3 * DT], hf0_sb[:, 0:1]
        )
        if phases == 99:
            # timing-model variant: skip the collective (TimelineSim
            # cannot model collectives); copy stage -> red locally
            nc.vector.tensor_copy(red[:], stage[:])
        else:
            cc_in = dram.tile([128, 3 * DT], F32)
            cc_out = dram.tile([128, 3 * DT], F32)
            nc.sync.dma_start(cc_in[:], stage[:])
            nc.gpsimd.collective_compute(
                "AllReduce", OP.add,
                replica_groups=[[0, 1], [2, 3], [4, 5], [6, 7]],
                ins=[cc_in.opt()], outs=[cc_out.opt()],
            )
            nc.sync.dma_start(red[:], cc_out[:])

        # ---------------- phase C: O,G -> raw cumsum -> L -> R ----------------
        # cumsum(P) = glob * (cumsum(O + b_o) + hf1*(W_O@sx + T b_o)); the
        # O/G matmuls + scan depend only on weights/x, so the collective
        # overlaps with the first `slack` blocks of that work.
        with (
            tc.tile_pool(name="wog", bufs=1) as wog_pool,
            tc.tile_pool(name="wo2", bufs=1) as wo2_pool,
            tc.tile_pool(name="xc", bufs=2) as xc_pool,
            tc.tile_pool(name="blkb", bufs=1) as blk_pool,
            tc.tile_pool(name="psc", bufs=8, space="PSUM") as psc,
        ):
            # block-0 x tiles first (the O matmuls need them right away and
            # woo is already resident), then W_G, then W_out — each needed
            # progressively later in the first C blocks
            xt_pre = xc_pool.tile([128, KT, CBLK], F32R, tag="xc", name="xt_pre")
            for kc in range(KT):
                nc.sync.dma_start(
                    xt_pre[:, kc, :],
                    x[kc * 128:(kc + 1) * 128, 1:1 + CBLK].bitcast(F32R),
                )
            wog = wog_pool.tile([128, KT, DT, 128], F32R)
            for kc in range(KT):
                nc.sync.dma_start(
                    wog[:, kc, :, :],
                    wqt[kc * 128:(kc + 1) * 128, 2 * D:3 * D]
                    .rearrange("p (a m) -> p a m", m=128).bitcast(F32R),
                )
            wo2 = wo2_pool.tile([128, KT, DT, 128], F32R)
            for kc in range(KT):
                nc.sync.dma_start(
                    wo2[:, kc, :, :],
                    wot[kc * 128:(kc + 1) * 128, :]
                    .rearrange("p (a m) -> p a m", m=128).bitcast(F32R),
                )

            # T * b_o column, needed by the offset tail (no allreduce dep)
            bo_t = cols.tile([128, DT], F32)
            nc.vector.tensor_scalar_mul(bo_t[:], bo_sb[:], float(T))

            nblk = T // CBLK
            if 50 <= phases < 99:
                nblk = phases - 50
            slack = min(SLACK, max(nblk - 1, 0))

            def c_og_part(blk, c_prev, xt=None):
                t0 = blk * CBLK
                if xt is None:
                    xt = xc_pool.tile([128, KT, CBLK], F32R, tag="xc", name="xt")
                    for kc in range(KT):
                        nc.sync.dma_start(
                            xt[:, kc, :],
                            x[kc * 128:(kc + 1) * 128, t0 + 1:t0 + 1 + CBLK]
                            .bitcast(F32R),
                        )
                pt = blk_pool.tile([128, DT, CBLK], F32, tag="pt", bufs=2,
                                   name="pt")
                ct = blk_pool.tile([128, DT, CBLK], F32, tag="ct",
                                   bufs=slack + 1, name="ct")
                gt = blk_pool.tile([128, DT, CBLK], F32, tag="gt",
                                   bufs=slack + 1, name="gt")
                carry = xc_pool.tile([128, DT], F32, tag="carry", name="carry")
                for a in range(DT):
                    ps = psc.tile([128, CBLK], F32, tag="ps", name="ps")
                    for kc in range(KT):
                        nc.tensor.matmul(
                            ps[:], woo[:, kc, a, :], xt[:, kc, :],
                            start=(kc == 0), stop=(kc == KT - 1),
                        )
                    # pt = O + b_o (glob applied later, after the allreduce)
                    nc.scalar.activation(
                        pt[:, a, :], ps[:], AF.Identity, bias=bo_sb[:, a:a + 1]
                    )
                    init = (zcol[:, 0:1] if c_prev is None
                            else c_prev[:, a:a + 1])
                    nc.vector.tensor_tensor_scan(
                        ct[:, a, :], pt[:, a, :], pt[:, a, :], init,
                        OP.add, OP.bypass,
                    )
                # carry the last cumsum column via ACT so the next
                # block's scan does not read a scan output directly
                nc.scalar.copy(carry[:], ct[:, :, CBLK - 1:CBLK])
                for a in range(DT):
                    ps = psc.tile([128, CBLK], F32, tag="ps", name="ps")
                    for kc in range(KT):
                        nc.tensor.matmul(
                            ps[:], wog[:, kc, a, :], xt[:, kc, :],
                            start=(kc == 0), stop=(kc == KT - 1),
                        )
                    sig = xc_pool.tile([128, CBLK], F32, tag="sig", name="sig")
                    nc.scalar.activation(
                        sig[:], ps[:], AF.Sigmoid, bias=bg_sb[:, a:a + 1]
                    )
                    nc.vector.scalar_tensor_tensor(
                        gt[:, a, :], ps[:], bg_sb[:, a:a + 1], sig[:],
                        OP.add, OP.mult,
                    )
                return ct, gt, carry

            def allreduce_tail():
                # everything downstream of `red`; emitted after `slack`
                # blocks of O/G work so the engine queues stay busy while
                # the collective completes
                recip = cols.tile([128, DT], F32, name="recip")
                nc.vector.reciprocal(recip[:], red[:, 0:DT])
                nc.vector.tensor_mul(glob[:], red[:, DT:2 * DT], recip[:])
                # offh = hf1 * (W_O @ sx_h0 + T*b_o)
                # (plain fp32 matmul: fp32r rejects a size-1 moving operand)
                for a in range(DT):
                    ps = psc.tile([128, CBLK], F32, tag="ps", name="ps")
                    for kc in range(KT):
                        nc.tensor.matmul(
                            ps[:, 0:1], woo[:, kc, a, :].bitcast(F32),
                            red[:, 2 * DT + kc:2 * DT + kc + 1],
                            start=(kc == 0), stop=(kc == KT - 1),
                        )
                    nc.vector.tensor_scalar(
                        offh[:, a:a + 1], ps[:, 0:1], bo_t[:, a:a + 1],
                        hf1_sb[:, 0:1], OP.add, OP.mult,
                    )

            def c_w_part(blk, ct, gt):
                t0 = blk * CBLK
                lt = blk_pool.tile([128, DT, CBLK], F32R, tag="lt", bufs=2,
                                   name="lt")
                rt = blk_pool.tile([128, DT, CBLK], F32, tag="rt", bufs=1,
                                   name="rt")
                for a in range(DT):
                    # full cumsum = glob * (raw cumsum + offset column)
                    nc.vector.tensor_scalar(
                        ct[:, a, :], ct[:, a, :], offh[:, a:a + 1],
                        glob[:, a:a + 1], OP.add, OP.mult,
                    )
                    nc.vector.tensor_mul(lt[:, a, :], gt[:, a, :], ct[:, a, :])
                for a in range(DT):
                    ps = psc.tile([128, CBLK], F32, tag="ps", name="ps")
                    for kc in range(KT):
                        nc.tensor.matmul(
                            ps[:], wo2[:, kc, a, :], lt[:, kc, :],
                            start=(kc == 0), stop=(kc == KT - 1),
                        )
                    nc.scalar.activation(
                        rt[:, a, :], ps[:], AF.Identity,
                        bias=bout_sb[:, a:a + 1],
                    )
                    nc.sync.dma_start(
                        r_out[a * 128:(a + 1) * 128, t0:t0 + CBLK],
                        rt[:, a, :],
                    )

            if phases == 4:
                allreduce_tail()
                nc.sync.dma_start(r_out[0:128, 0:DT], offh[:])
                nc.sync.dma_start(r_out[128:256, 0:DT], glob[:])
                return

            pending = []
            c_prev = None
            tail_done = False
            for blk in range(nblk):
                ct, gt, carry = c_og_part(blk, c_prev,
                                          xt=xt_pre if blk == 0 else None)
                c_prev = carry
                pending.append((blk, ct, gt))
                if blk >= slack:
                    if not tail_done:
                        allreduce_tail()
                        tail_done = True
                    b2, ct2, gt2 = pending.pop(0)
                    c_w_part(b2, ct2, gt2)
            if not tail_done:
                allreduce_tail()
            for b2, ct2, gt2 in pending:
                c_w_part(b2, ct2, gt2)


_CACHE = {}


def _build(phases=5):
    if phases in _CACHE:
        return _CACHE[phases]
    nc = bacc.Bacc(None, target_bir_lowering=False, num_devices=N_CORES)
    prm = {
        "x": nc.declare_dram_parameter("x", [DM, TH], F32, isOutput=False),
        "wqt": nc.declare_dram_parameter("wqt", [DM, 3 * D], F32, isOutput=False),
        "wct": nc.declare_dram_parameter("wct", [3, D, D], F32, isOutput=False),
        "wat": nc.declare_dram_parameter("wat", [D, D], F32, isOutput=False),
        "wot": nc.declare_dram_parameter("wot", [D, D], F32, isOutput=False),
        "bq": nc.declare_dram_parameter("bq", [128, DT], F32, isOutput=False),
        "bo": nc.declare_dram_parameter("bo", [128, DT], F32, isOutput=False),
        "bg": nc.declare_dram_parameter("bg", [128, DT], F32, isOutput=False),
        "cb": nc.declare_dram_parameter("cb", [128, DT], F32, isOutput=False),
        "bout": nc.declare_dram_parameter("bout", [128, DT], F32, isOutput=False),
        "hf0": nc.declare_dram_parameter("hf0", [128, 1], F32, isOutput=False),
        "hf1": nc.declare_dram_parameter("hf1", [128, 1], F32, isOutput=False),
        "r": nc.declare_dram_parameter("r", [DM, T], F32, isOutput=True),
    }
    with tile.TileContext(nc, num_cores=N_CORES) as tc:
        _emit(tc, nc, prm, phases)
    nc.compile()
    _CACHE[phases] = nc
    return nc


def make_in_maps(x, W_qog, b_qog, conv_w, conv_b, w_a, W_out, b_out):
    f = np.float32
    x = np.asarray(x, f)
    wqt = np.ascontiguousarray(np.asarray(W_qog, f).T)          # [dm, 3d]
    wct = np.ascontiguousarray(np.asarray(conv_w, f).transpose(2, 1, 0))
    wat = np.ascontiguousarray(np.asarray(w_a, f).T)
    wot = np.ascontiguousarray(np.asarray(W_out, f).T)

    def col(v):  # [d] -> [128, DT] with d = a*128 + p
        return np.ascontiguousarray(np.asarray(v, f).reshape(DT, 128).T)

    b_qog = np.asarray(b_qog, f)
    bq, bo, bg = col(b_qog[:D]), col(b_qog[D:2 * D]), col(b_qog[2 * D:])
    cb, bout = col(conv_b), col(b_out)

    in_maps = []
    for c in range(N_CORES):
        b, h = c // 2, c % 2
        t0 = h * T
        xs = np.zeros((TH, DM), f)
        xs[1:T + 1] = x[b, t0:t0 + T]
        if t0 > 0:
            xs[0] = x[b, t0 - 1]
        if t0 + T < S:
            xs[T + 1] = x[b, t0 + T]
        xs = np.ascontiguousarray(xs.T)            # [DM, TH] feature-major
        in_maps.append({
            "x": xs, "wqt": wqt, "wct": wct, "wat": wat, "wot": wot,
            "bq": bq, "bo": bo, "bg": bg, "cb": cb, "bout": bout,
            "hf0": np.full((128, 1), 1.0 - h, f),
            "hf1": np.full((128, 1), float(h), f),
        })
    return in_maps


def kernel(x, W_qog, b_qog, conv_w, conv_b, w_a, W_out, b_out):
    nc = _build(5)
    in_maps = make_in_maps(x, W_qog, b_qog, conv_w, conv_b, w_a, W_out, b_out)
    res = None
    for attempt in range(3):
        try:
            res = run_bass_kernel_spmd(nc, in_maps, list(range(N_CORES)))
            break
        except Exception:
            # the execution path through the device bridge is occasionally
            # flaky (worker hangup); reset the backend and retry
            if attempt == 2:
                raise
            import jax

            try:
                jax.clear_backends()
            except Exception:
                pass
            import time

            time.sleep(5)
    out = np.empty((B, S, DM), np.float32)
    for c in range(N_CORES):
        b, h = c // 2, c % 2
        out[b, h * T:(h + 1) * T, :] = res.results[c]["r"].T
    return out


# revision 3
# speedup vs baseline: 13.7476x; 1.2589x over previous
"""F2NetHead Trainium2 kernel (8 NeuronCores, Bass/Tile) — v2.

Reference computation (per batch b):
    qog = x @ W_qog.T + b_qog ; Q,O,G = split(qog)
    cq  = silu(conv1d(Q, conv_w, pad=1) + conv_b)          # mixes channels
    l   = (cq @ w_a.T) / sqrt(d)
    attn= softmax(l, axis=seq)
    glob= sum_seq(Q * attn)                                 # [1, d]
    P   = O * glob
    L   = silu(G) * cumsum(P, axis=seq)
    R   = L @ W_out.T + b_out

Sharding: 8 cores = 4 batches x 2 sequence halves; x arrives with a
1-token halo so the conv needs no neighbor exchange. Per batch pair the
only comm is an AllReduce of 3 [d] vectors: E = sum exp(l),
N = sum Q*exp(l), sx = sum of half-0's x rows.

v2 changes vs v1:
  *  glob factors out of the cumsum:  cumsum(P) = glob * cumsum(O + b_o)
     and the half-1 offset = glob * (W_O @ sx + T b_o).  Phase C's O/G
     matmuls and the raw cumsum therefore have NO dependency on the
     collective; only a per-block tensor_scalar (+offset, *glob) and the
     W_out matmul consume the allreduce result.  SLACK blocks of O/G
     work are emitted before the first allreduce-dependent instruction
     on each engine queue, hiding the collective latency.
  *  weight DMA is staged: W_Q before phase A, conv weights + w_a during
     B1, W_O during B2, W_G/W_out at C start — removes the 8 MiB
     startup stall and the B1->B2 weight-load gap.

On-chip layout is feature-major ([d partitions, tokens free]) so every
sequence-axis op (softmax sums, global sum, cumsum) is a free-dim op.
All matmuls run in float32r (full PE rate); silu is x*sigmoid(x).
"""

import numpy as np

import concourse.bacc as bacc
import concourse.mybir as mybir
import concourse.tile as tile
from concourse.bass_utils import run_bass_kernel_spmd

F32 = mybir.dt.float32
F32R = mybir.dt.float32r
AF = mybir.ActivationFunctionType
OP = mybir.AluOpType

B, S, D, DM = 4, 4096, 1024, 1024
N_CORES = 8
T = S // 2            # tokens per core
TH = T + 2            # with halo
DT = D // 128         # d tiles (8)
KT = DM // 128        # contraction tiles (8)
ABLK = 410            # phase A token block (5 blocks over TH=2050)
BBLK = 512            # phase B token block (4 blocks over T)
CBLK = 256            # phase C token block (8 blocks over T)
SLACK = 2             # C blocks of O/G work emitted ahead of any
                      # allreduce-dependent instruction
SCALE = 1.0 / float(np.sqrt(D))


def _emit(tc, nc, prm, phases=5):
    reps = 1
    if phases >= 100:
        reps, phases = phases // 100, 5
    for _ in range(reps):
        _emit_once(tc, nc, prm, phases)


def _emit_once(tc, nc, prm, phases):
    x, wqt, wct, wat, wot = prm["x"], prm["wqt"], prm["wct"], prm["wat"], prm["wot"]
    bq, bo, bg, cb, bout = prm["bq"], prm["bo"], prm["bg"], prm["cb"], prm["bout"]
    hf0, hf1, r_out = prm["hf0"], prm["hf1"], prm["r"]

    with (
        tc.tile_pool(name="cols", bufs=1) as cols,
        tc.tile_pool(name="woo", bufs=1) as woo_pool,
        tc.tile_pool(name="dram", bufs=1, space="DRAM") as dram,
    ):
        # per-partition bias / flag columns ([128, DT] with d = a*128 + p)
        bq_sb = cols.tile([128, DT], F32)
        bo_sb = cols.tile([128, DT], F32)
        bg_sb = cols.tile([128, DT], F32)
        cb_sb = cols.tile([128, DT], F32)
        bout_sb = cols.tile([128, DT], F32)
        hf0_sb = cols.tile([128, 1], F32)
        hf1_sb = cols.tile([128, 1], F32)
        for t_, d_ in ((bq_sb, bq), (bo_sb, bo), (bg_sb, bg), (cb_sb, cb),
                       (bout_sb, bout), (hf0_sb, hf0), (hf1_sb, hf1)):
            nc.sync.dma_start(t_[:], d_[:])

        # accumulators that survive across phases
        sx_cols = cols.tile([128, KT, 5], F32)      # per-A-block x sums
        e_cols = cols.tile([128, DT * 4], F32)      # per-(a,B-block) exp sums
        n_cols = cols.tile([128, DT * 4], F32)      # per-(a,B-block) Q*exp sums
        stage = cols.tile([128, 3 * DT], F32)       # allreduce staging
        red = cols.tile([128, 3 * DT], F32)         # allreduce result
        glob = cols.tile([128, DT], F32)
        offh = cols.tile([128, DT], F32)            # hf1*(W_O@sx + T b_o)
        zcol = cols.tile([128, 1], F32)             # scan init for block 0
        nc.vector.memset(zcol[:], 0.0)

        # ---------------- phase A: Q^T over TH halo'd tokens ----------------
        with tc.tile_pool(name="qt", bufs=1) as qt_pool:
            qt = qt_pool.tile([128, DT, TH], F32R)
            with (
                tc.tile_pool(name="wq", bufs=1) as wq_pool,
                tc.tile_pool(name="xa", bufs=2) as xa_pool,
                tc.tile_pool(name="psa", bufs=8, space="PSUM") as psa,
            ):
                wq = [wq_pool.tile([128, DT, 128], F32R, tag=f"wq{kc}",
                                   name=f"wq{kc}") for kc in range(KT)]
                # block-0 x tiles interleave with the wq loads so the first
                # contraction chain can start as soon as pair 0 lands
                xt0 = [xa_pool.tile([128, ABLK], F32R, tag=f"xa{kc}",
                                    name=f"xa{kc}") for kc in range(KT)]
                for kc in range(KT):
                    nc.sync.dma_start(
                        wq[kc][:],
                        wqt[kc * 128:(kc + 1) * 128, 0:D]
                        .rearrange("p (a m) -> p a m", m=128).bitcast(F32R),
                    )
                    nc.sync.dma_start(
                        xt0[kc][:],
                        x[kc * 128:(kc + 1) * 128, 0:ABLK].bitcast(F32R),
                    )
                for blk in range(5):
                    t0 = blk * ABLK
                    if blk == 0:
                        xt = xt0
                    else:
                        xt = [xa_pool.tile([128, ABLK], F32R, tag=f"xa{kc}",
                                           name=f"xa{kc}") for kc in range(KT)]
                        for kc in range(KT):
                            nc.sync.dma_start(
                                xt[kc][:],
                                x[kc * 128:(kc + 1) * 128, t0:t0 + ABLK]
                                .bitcast(F32R),
                            )
                    # x column-sums over main (non-halo) tokens for cumsum offset
                    lo = 1 - t0 if t0 < 1 else 0
                    hi = ABLK - max(0, t0 + ABLK - (TH - 1))
                    for kc in range(KT):
                        nc.vector.tensor_reduce(
                            sx_cols[:, kc, blk:blk + 1], xt[kc][:, lo:hi],
                            axis=mybir.AxisListType.X, op=OP.add,
                        )
                    for a in range(DT):
                        ps = psa.tile([128, ABLK], F32, tag="ps")
                        for kc in range(KT):
                            nc.tensor.matmul(
                                ps[:], wq[kc][:, a, :], xt[kc][:],
                                start=(kc == 0), stop=(kc == KT - 1),
                            )
                        nc.vector.tensor_scalar_add(
                            qt[:, a, t0:t0 + ABLK], ps[:], bq_sb[:, a:a + 1]
                        )

            if phases == 1:
                for a in range(DT):
                    nc.sync.dma_start(
                        r_out[a * 128:(a + 1) * 128, 0:T],
                        qt[:, a, 1:T + 1].bitcast(F32),
                    )
                return

            # ------------- phase B1: cq^T = silu(conv(Q)) -------------
            with tc.tile_pool(name="cq", bufs=1) as cq_pool:
                cq = cq_pool.tile([128, DT, T], F32R)
                with (
                    tc.tile_pool(name="wc", bufs=2) as wc_pool,
                    tc.tile_pool(name="psb", bufs=8, space="PSUM") as psb,
                ):
                    for a in range(DT):
                        wc = wc_pool.tile([128, 3, KT, 128], F32R, tag="wc")
                        for k3 in range(3):
                            nc.sync.dma_start(
                                wc[:, k3, :, :],
                                wct[k3, :, a * 128:(a + 1) * 128]
                                .rearrange("(kc p) m -> p kc m", p=128)
                                .bitcast(F32R),
                            )
                        for blk in range(T // BBLK):
                            t0 = blk * BBLK
                            ps = psb.tile([128, BBLK], F32, tag="ps")
                            first = True
                            for k3 in range(3):
                                for kc in range(KT):
                                    nc.tensor.matmul(
                                        ps[:], wc[:, k3, kc, :],
                                        qt[:, kc, t0 + k3:t0 + k3 + BBLK],
                                        start=first,
                                        stop=(k3 == 2 and kc == KT - 1),
                                    )
                                    first = False
                            sig = wc_pool.tile([128, BBLK], F32, tag="sig")
                            nc.scalar.activation(
                                sig[:], ps[:], AF.Sigmoid, bias=cb_sb[:, a:a + 1]
                            )
                            nc.vector.scalar_tensor_tensor(
                                cq[:, a, t0:t0 + BBLK], ps[:], cb_sb[:, a:a + 1],
                                sig[:], OP.add, OP.mult,
                            )

                if phases == 2:
                    for a in range(DT):
                        nc.sync.dma_start(
                            r_out[a * 128:(a + 1) * 128, 0:T],
                            cq[:, a, :].bitcast(F32),
                        )
                    return

                # ------- phase B2: E/N partial sums from exp(logits) -------
                # w_a loads first (needed immediately); W_O^T queued behind
                # it, streaming during B2 compute (offset matvec + phase C)
                with (
                    tc.tile_pool(name="wa", bufs=1) as wa_pool,
                    tc.tile_pool(name="ex", bufs=2) as ex_pool,
                    tc.tile_pool(name="psl", bufs=8, space="PSUM") as psl,
                ):
                    wa = [wa_pool.tile([128, DT, 128], F32R, tag=f"wa{kc}",
                                       name=f"wa{kc}") for kc in range(KT)]
                    for kc in range(KT):
                        nc.sync.dma_start(
                            wa[kc][:],
                            wat[kc * 128:(kc + 1) * 128, :]
                            .rearrange("p (a m) -> p a m", m=128).bitcast(F32R),
                        )
                    woo = woo_pool.tile([128, KT, DT, 128], F32R)
                    for kc in range(KT):
                        nc.sync.dma_start(
                            woo[:, kc, :, :],
                            wqt[kc * 128:(kc + 1) * 128, D:2 * D]
                            .rearrange("p (a m) -> p a m", m=128).bitcast(F32R),
                        )
                    for blk in range(T // BBLK):
                        t0 = blk * BBLK
                        for a in range(DT):
                            ps = psl.tile([128, BBLK], F32, tag="ps")
                            for kc in range(KT):
                                nc.tensor.matmul(
                                    ps[:], wa[kc][:, a, :],
                                    cq[:, kc, t0:t0 + BBLK],
                                    start=(kc == 0), stop=(kc == KT - 1),
                                )
                            expl = ex_pool.tile([128, BBLK], F32, tag="expl")
                            idx = a * 4 + blk
                            nc.scalar.activation(
                                expl[:], ps[:], AF.Exp, scale=SCALE,
                                accum_out=e_cols[:, idx:idx + 1],
                            )
                            prod = ex_pool.tile([128, BBLK], F32, tag="prod")
                            nc.vector.scalar_tensor_tensor(
                                prod[:], expl[:], 0.0,
                                qt[:, a, t0 + 1:t0 + 1 + BBLK].bitcast(F32),
                                OP.add, OP.mult,
                                accum_out=n_cols[:, idx:idx + 1],
                            )

        if phases == 3:
            nc.sync.dma_start(r_out[0:128, 0:32].rearrange("p t -> p t"), e_cols[:])
            nc.sync.dma_start(r_out[128:256, 0:32], n_cols[:])
            return

        # ------------- allreduce E, N, offv over the seq pair -------------
        nc.vector.tensor_reduce(
            stage[:, 0:DT], e_cols[:].rearrange("p (a b) -> p a b", b=4),
            axis=mybir.AxisListType.X, op=OP.add,
        )
        nc.vector.tensor_reduce(
            stage[:, DT:2 * DT], n_cols[:].rearrange("p (a b) -> p a b", b=4),
            axis=mybir.AxisListType.X, op=OP.add,
        )
        # offv = W_O @ (own x sums), masked to the first half: each core
        # computes the matvec from its LOCAL sx before the collective (PE is
        # otherwise idle here), so nothing downstream of the allreduce needs
        # the PE — the post-collective tail is pure DVE scalar work
        sx = cols.tile([128, KT], F32, name="sx")
        nc.vector.tensor_reduce(
            sx[:], sx_cols[:], axis=mybir.AxisListType.X, op=OP.add,
        )
        with tc.tile_pool(name="psv", bufs=2, space="PSUM") as psv:
            for a in range(DT):
                ps = psv.tile([128, 1], F32, tag="ps", name="ps")
                for kc in range(KT):
                    nc.tensor.matmul(
                        ps[:], woo[:, kc, a, :].bitcast(F32),
                        sx[:, kc:kc + 1],
                        start=(kc == 0), stop=(kc == KT - 1),
                    )
                nc.vector.tensor_scalar_mul(
                    stage[:, 2 * DT + a:2 * DT + a + 1], ps[:], hf0_sb[:, 0:1]
                )
        if phases == 99:
            # timing-model variant: skip the collective (TimelineSim
            # cannot model collectives); copy stage -> red locally
            nc.vector.tensor_copy(red[:], stage[:])
        else:
            cc_in = dram.tile([128, 3 * DT], F32)
            cc_out = dram.tile([128, 3 * DT], F32)
            nc.sync.dma_start(cc_in[:], stage[:])
            nc.gpsimd.collective_compute(
                "AllReduce", OP.add,
                replica_groups=[[0, 1], [2, 3], [4, 5], [6, 7]],
                ins=[cc_in.opt()], outs=[cc_out.opt()],
            )
            nc.sync.dma_start(red[:], cc_out[:])

        # ---------------- phase C: O,G -> raw cumsum -> L -> R ----------------
        # cumsum(P) = glob * (cumsum(O + b_o) + hf1*(W_O@sx + T b_o)); the
        # O/G matmuls + scan depend only on weights/x, so the collective
        # overlaps with the first `slack` blocks of that work.
        with (
            tc.tile_pool(name="wog", bufs=1) as wog_pool,
            tc.tile_pool(name="wo2", bufs=1) as wo2_pool,
            tc.tile_pool(name="xc", bufs=2) as xc_pool,
            tc.tile_pool(name="blkb", bufs=1) as blk_pool,
            tc.tile_pool(name="psc", bufs=8, space="PSUM") as psc,
        ):
            # block-0 x tiles first (the O matmuls need them right away and
            # woo is already resident), then W_G, then W_out — each needed
            # progressively later in the first C blocks
            xt_pre = xc_pool.tile([128, KT, CBLK], F32R, tag="xc", name="xt_pre")
            for kc in range(KT):
                nc.sync.dma_start(
                    xt_pre[:, kc, :],
                    x[kc * 128:(kc + 1) * 128, 1:1 + CBLK].bitcast(F32R),
                )
            wog = wog_pool.tile([128, KT, DT, 128], F32R)
            for kc in range(KT):
                nc.sync.dma_start(
                    wog[:, kc, :, :],
                    wqt[kc * 128:(kc + 1) * 128, 2 * D:3 * D]
                    .rearrange("p (a m) -> p a m", m=128).bitcast(F32R),
                )
            wo2 = wo2_pool.tile([128, KT, DT, 128], F32R)
            for kc in range(KT):
                nc.sync.dma_start(
                    wo2[:, kc, :, :],
                    wot[kc * 128:(kc + 1) * 128, :]
                    .rearrange("p (a m) -> p a m", m=128).bitcast(F32R),
                )

            # hf1 * T * b_o, needed by the offset tail (no allreduce dep)
            bo_th = cols.tile([128, DT], F32)
            nc.vector.tensor_scalar(
                bo_th[:], bo_sb[:], float(T), hf1_sb[:, 0:1], OP.mult, OP.mult,
            )

            nblk = T // CBLK
            if 50 <= phases < 99:
                nblk = phases - 50
            slack = min(SLACK, max(nblk - 1, 0))

            def c_og_part(blk, c_prev, xt=None):
                t0 = blk * CBLK
                if xt is None:
                    xt = xc_pool.tile([128, KT, CBLK], F32R, tag="xc", name="xt")
                    for kc in range(KT):
                        nc.sync.dma_start(
                            xt[:, kc, :],
                            x[kc * 128:(kc + 1) * 128, t0 + 1:t0 + 1 + CBLK]
                            .bitcast(F32R),
                        )
                pt = blk_pool.tile([128, DT, CBLK], F32, tag="pt", bufs=2,
                                   name="pt")
                ct = blk_pool.tile([128, DT, CBLK], F32, tag="ct",
                                   bufs=slack + 1, name="ct")
                gt = blk_pool.tile([128, DT, CBLK], F32, tag="gt",
                                   bufs=slack + 1, name="gt")
                carry = xc_pool.tile([128, DT], F32, tag="carry", name="carry")
                for a in range(DT):
                    ps = psc.tile([128, CBLK], F32, tag="ps", name="ps")
                    for kc in range(KT):
                        nc.tensor.matmul(
                            ps[:], woo[:, kc, a, :], xt[:, kc, :],
                            start=(kc == 0), stop=(kc == KT - 1),
                        )
                    # pt = O + b_o (glob applied later, after the allreduce)
                    nc.scalar.activation(
                        pt[:, a, :], ps[:], AF.Identity, bias=bo_sb[:, a:a + 1]
                    )
                    init = (zcol[:, 0:1] if c_prev is None
                            else c_prev[:, a:a + 1])
                    nc.vector.tensor_tensor_scan(
                        ct[:, a, :], pt[:, a, :], pt[:, a, :], init,
                        OP.add, OP.bypass,
                    )
                # carry the last cumsum column via ACT so the next
                # block's scan does not read a scan output directly
                nc.scalar.copy(carry[:], ct[:, :, CBLK - 1:CBLK])
                for a in range(DT):
                    ps = psc.tile([128, CBLK], F32, tag="ps", name="ps")
                    for kc in range(KT):
                        nc.tensor.matmul(
                            ps[:], wog[:, kc, a, :], xt[:, kc, :],
                            start=(kc == 0), stop=(kc == KT - 1),
                        )
                    sig = xc_pool.tile([128, CBLK], F32, tag="sig", name="sig")
                    nc.scalar.activation(
                        sig[:], ps[:], AF.Sigmoid, bias=bg_sb[:, a:a + 1]
                    )
                    nc.vector.scalar_tensor_tensor(
                        gt[:, a, :], ps[:], bg_sb[:, a:a + 1], sig[:],
                        OP.add, OP.mult,
                    )
                return ct, gt, carry

            def allreduce_tail():
                # everything downstream of `red`; emitted after `slack`
                # blocks of O/G work so the engine queues stay busy while
                # the collective completes
                recip = cols.tile([128, DT], F32, name="recip")
                nc.vector.reciprocal(recip[:], red[:, 0:DT])
                nc.vector.tensor_mul(glob[:], red[:, DT:2 * DT], recip[:])
                # R = (gt*(ct0+offh)) @ (W_out^T scaled by glob along the
                # contraction): fold glob into wo2's columns once instead of
                # scaling every C block (glob is constant over tokens).
                # DVE only: the f32r-consumed output needs DVE's f32r rounding.
                for kc in range(KT):
                    nc.vector.tensor_scalar_mul(
                        wo2[:, kc, :, :],
                        wo2[:, kc, :, :].bitcast(F32), glob[:, kc:kc + 1],
                    )
                # offh = hf1*offv + hf1*T*b_o; offv arrived via the allreduce
                nc.vector.scalar_tensor_tensor(
                    offh[:], red[:, 2 * DT:3 * DT], hf1_sb[:, 0:1], bo_th[:],
                    OP.mult, OP.add,
                )

            def c_w_part(blk, ct, gt):
                t0 = blk * CBLK
                lt = blk_pool.tile([128, DT, CBLK], F32R, tag="lt", bufs=2,
                                   name="lt")
                rt = blk_pool.tile([128, DT, CBLK], F32, tag="rt", bufs=1,
                                   name="rt")
                for a in range(DT):
                    # lt = gt * (raw cumsum + offset column); glob lives in wo2
                    nc.vector.scalar_tensor_tensor(
                        lt[:, a, :], ct[:, a, :], offh[:, a:a + 1],
                        gt[:, a, :], OP.add, OP.mult,
                    )
                for a in range(DT):
                    ps = psc.tile([128, CBLK], F32, tag="ps", name="ps")
                    for kc in range(KT):
                        nc.tensor.matmul(
                            ps[:], wo2[:, kc, a, :], lt[:, kc, :],
                            start=(kc == 0), stop=(kc == KT - 1),
                        )
                    nc.scalar.activation(
                        rt[:, a, :], ps[:], AF.Identity,
                        bias=bout_sb[:, a:a + 1],
                    )
                    nc.sync.dma_start(
                        r_out[a * 128:(a + 1) * 128, t0:t0 + CBLK],
                        rt[:, a, :],
                    )

            if phases == 4:
                allreduce_tail()
                nc.sync.dma_start(r_out[0:128, 0:DT], offh[:])
                nc.sync.dma_start(r_out[128:256, 0:DT], glob[:])
                return

            pending = []
            c_prev = None
            tail_done = False
            for blk in range(nblk):
                ct, gt, carry = c_og_part(blk, c_prev,
                                          xt=xt_pre if blk == 0 else None)
                c_prev = carry
                pending.append((blk, ct, gt))
                if blk >= slack:
                    if not tail_done:
                        allreduce_tail()
                        tail_done = True
                    b2, ct2, gt2 = pending.pop(0)
                    c_w_part(b2, ct2, gt2)
            if not tail_done:
                allreduce_tail()
            for b2, ct2, gt2 in pending:
                c_w_part(b2, ct2, gt2)


_CACHE = {}


def _build(phases=5):
    if phases in _CACHE:
        return _CACHE[phases]
    nc = bacc.Bacc(None, target_bir_lowering=False, num_devices=N_CORES)
    prm = {
        "x": nc.declare_dram_parameter("x", [DM, TH], F32, isOutput=False),
        "wqt": nc.declare_dram_parameter("wqt", [DM, 3 * D], F32, isOutput=False),
        "wct": nc.declare_dram_parameter("wct", [3, D, D], F32, isOutput=False),
        "wat": nc.declare_dram_parameter("wat", [D, D], F32, isOutput=False),
        "wot": nc.declare_dram_parameter("wot", [D, D], F32, isOutput=False),
        "bq": nc.declare_dram_parameter("bq", [128, DT], F32, isOutput=False),
        "bo": nc.declare_dram_parameter("bo", [128, DT], F32, isOutput=False),
        "bg": nc.declare_dram_parameter("bg", [128, DT], F32, isOutput=False),
        "cb": nc.declare_dram_parameter("cb", [128, DT], F32, isOutput=False),
        "bout": nc.declare_dram_parameter("bout", [128, DT], F32, isOutput=False),
        "hf0": nc.declare_dram_parameter("hf0", [128, 1], F32, isOutput=False),
        "hf1": nc.declare_dram_parameter("hf1", [128, 1], F32, isOutput=False),
        "r": nc.declare_dram_parameter("r", [DM, T], F32, isOutput=True),
    }
    with tile.TileContext(nc, num_cores=N_CORES) as tc:
        _emit(tc, nc, prm, phases)
    nc.compile()
    _CACHE[phases] = nc
    return nc


def make_in_maps(x, W_qog, b_qog, conv_w, conv_b, w_a, W_out, b_out):
    f = np.float32
    x = np.asarray(x, f)
    wqt = np.ascontiguousarray(np.asarray(W_qog, f).T)          # [dm, 3d]
    wct = np.ascontiguousarray(np.asarray(conv_w, f).transpose(2, 1, 0))
    wat = np.ascontiguousarray(np.asarray(w_a, f).T)
    wot = np.ascontiguousarray(np.asarray(W_out, f).T)

    def col(v):  # [d] -> [128, DT] with d = a*128 + p
        return np.ascontiguousarray(np.asarray(v, f).reshape(DT, 128).T)

    b_qog = np.asarray(b_qog, f)
    bq, bo, bg = col(b_qog[:D]), col(b_qog[D:2 * D]), col(b_qog[2 * D:])
    cb, bout = col(conv_b), col(b_out)

    in_maps = []
    for c in range(N_CORES):
        b, h = c // 2, c % 2
        t0 = h * T
        xs = np.zeros((TH, DM), f)
        xs[1:T + 1] = x[b, t0:t0 + T]
        if t0 > 0:
            xs[0] = x[b, t0 - 1]
        if t0 + T < S:
            xs[T + 1] = x[b, t0 + T]
        xs = np.ascontiguousarray(xs.T)            # [DM, TH] feature-major
        in_maps.append({
            "x": xs, "wqt": wqt, "wct": wct, "wat": wat, "wot": wot,
            "bq": bq, "bo": bo, "bg": bg, "cb": cb, "bout": bout,
            "hf0": np.full((128, 1), 1.0 - h, f),
            "hf1": np.full((128, 1), float(h), f),
        })
    return in_maps


def kernel(x, W_qog, b_qog, conv_w, conv_b, w_a, W_out, b_out):
    nc = _build(5)
    in_maps = make_in_maps(x, W_qog, b_qog, conv_w, conv_b, w_a, W_out, b_out)
    res = None
    for attempt in range(3):
        try:
            res = run_bass_kernel_spmd(nc, in_maps, list(range(N_CORES)))
            break
        except Exception:
            # the execution path through the device bridge is occasionally
            # flaky (worker hangup); reset the backend and retry
            if attempt == 2:
                raise
            import jax

            try:
                jax.clear_backends()
            except Exception:
                pass
            import time

            time.sleep(5)
    out = np.empty((B, S, DM), np.float32)
    for c in range(N_CORES):
        b, h = c // 2, c % 2
        out[b, h * T:(h + 1) * T, :] = res.results[c]["r"].T
    return out


# revision 4
# speedup vs baseline: 14.8337x; 1.0790x over previous
"""F2NetHead Trainium2 kernel (8 NeuronCores, Bass/Tile) — v2.

Reference computation (per batch b):
    qog = x @ W_qog.T + b_qog ; Q,O,G = split(qog)
    cq  = silu(conv1d(Q, conv_w, pad=1) + conv_b)          # mixes channels
    l   = (cq @ w_a.T) / sqrt(d)
    attn= softmax(l, axis=seq)
    glob= sum_seq(Q * attn)                                 # [1, d]
    P   = O * glob
    L   = silu(G) * cumsum(P, axis=seq)
    R   = L @ W_out.T + b_out

Sharding: 8 cores = 4 batches x 2 sequence halves; x arrives with a
1-token halo so the conv needs no neighbor exchange. Per batch pair the
only comm is an AllReduce of 3 [d] vectors: E = sum exp(l),
N = sum Q*exp(l), sx = sum of half-0's x rows.

v2 changes vs v1:
  *  glob factors out of the cumsum:  cumsum(P) = glob * cumsum(O + b_o)
     and the half-1 offset = glob * (W_O @ sx + T b_o).  Phase C's O/G
     matmuls and the raw cumsum therefore have NO dependency on the
     collective; only a per-block tensor_scalar (+offset, *glob) and the
     W_out matmul consume the allreduce result.  SLACK blocks of O/G
     work are emitted before the first allreduce-dependent instruction
     on each engine queue, hiding the collective latency.
  *  weight DMA is staged: W_Q before phase A, conv weights + w_a during
     B1, W_O during B2, W_G/W_out at C start — removes the 8 MiB
     startup stall and the B1->B2 weight-load gap.

On-chip layout is feature-major ([d partitions, tokens free]) so every
sequence-axis op (softmax sums, global sum, cumsum) is a free-dim op.
All matmuls run in float32r (full PE rate); silu is x*sigmoid(x).
"""

import numpy as np

import concourse.bacc as bacc
import concourse.mybir as mybir
import concourse.tile as tile
from concourse.bass_utils import run_bass_kernel_spmd

F32 = mybir.dt.float32
F32R = mybir.dt.float32r
AF = mybir.ActivationFunctionType
OP = mybir.AluOpType

B, S, D, DM = 4, 4096, 1024, 1024
N_CORES = 8
T = S // 2            # tokens per core
TH = T + 2            # with halo
DT = D // 128         # d tiles (8)
KT = DM // 128        # contraction tiles (8)
ABLK = 410            # phase A token block (5 blocks over TH=2050)
BBLK = 512            # phase B token block (4 blocks over T)
CBLK = 256            # phase C token block (8 blocks over T)
SLACK = 2             # C blocks of O/G work emitted ahead of any
                      # allreduce-dependent instruction
SCALE = 1.0 / float(np.sqrt(D))


def _emit(tc, nc, prm, phases=5):
    reps = 1
    if phases >= 100:
        reps, phases = phases // 100, 5
    for _ in range(reps):
        _emit_once(tc, nc, prm, phases)


def _emit_once(tc, nc, prm, phases):
    x, wqt, wct, wat, wot = prm["x"], prm["wqt"], prm["wct"], prm["wat"], prm["wot"]
    bq, bo, bg, cb, bout = prm["bq"], prm["bo"], prm["bg"], prm["cb"], prm["bout"]
    hf0, hf1, r_out = prm["hf0"], prm["hf1"], prm["r"]

    with (
        tc.tile_pool(name="cols", bufs=1) as cols,
        tc.tile_pool(name="woo", bufs=1) as woo_pool,
        tc.tile_pool(name="dram", bufs=1, space="DRAM") as dram,
    ):
        # per-partition bias / flag columns ([128, DT] with d = a*128 + p)
        bq_sb = cols.tile([128, DT], F32)
        bo_sb = cols.tile([128, DT], F32)
        bg_sb = cols.tile([128, DT], F32)
        cb_sb = cols.tile([128, DT], F32)
        bout_sb = cols.tile([128, DT], F32)
        hf0_sb = cols.tile([128, 1], F32)
        hf1_sb = cols.tile([128, 1], F32)
        for t_, d_ in ((bq_sb, bq), (bo_sb, bo), (bg_sb, bg), (cb_sb, cb),
                       (bout_sb, bout), (hf0_sb, hf0), (hf1_sb, hf1)):
            nc.sync.dma_start(t_[:], d_[:])

        # accumulators that survive across phases
        sx_cols = cols.tile([128, KT, 5], F32)      # per-A-block x sums
        e_cols = cols.tile([128, DT * 4], F32)      # per-(a,B-block) exp sums
        n_cols = cols.tile([128, DT * 4], F32)      # per-(a,B-block) Q*exp sums
        stage = cols.tile([128, 3 * DT], F32)       # allreduce staging
        red = cols.tile([128, 3 * DT], F32)         # allreduce result
        glob = cols.tile([128, DT], F32)
        offh = cols.tile([128, DT], F32)            # hf1*(W_O@sx + T b_o)
        zcol = cols.tile([128, 1], F32)             # scan init for block 0
        nc.vector.memset(zcol[:], 0.0)

        # ---------------- phase A: Q^T over TH halo'd tokens ----------------
        with tc.tile_pool(name="qt", bufs=1) as qt_pool:
            qt = qt_pool.tile([128, DT, TH], F32R)
            with (
                tc.tile_pool(name="wq", bufs=1) as wq_pool,
                tc.tile_pool(name="xa", bufs=2) as xa_pool,
                tc.tile_pool(name="psa", bufs=8, space="PSUM") as psa,
            ):
                wq = [wq_pool.tile([128, DT, 128], F32R, tag=f"wq{kc}",
                                   name=f"wq{kc}") for kc in range(KT)]
                # block-0 x tiles interleave with the wq loads so the first
                # contraction chain can start as soon as pair 0 lands
                xt0 = [xa_pool.tile([128, ABLK], F32R, tag=f"xa{kc}",
                                    name=f"xa{kc}") for kc in range(KT)]
                for kc in range(KT):
                    nc.sync.dma_start(
                        wq[kc][:],
                        wqt[kc * 128:(kc + 1) * 128, 0:D]
                        .rearrange("p (a m) -> p a m", m=128).bitcast(F32R),
                    )
                    nc.sync.dma_start(
                        xt0[kc][:],
                        x[kc * 128:(kc + 1) * 128, 0:ABLK].bitcast(F32R),
                    )
                for blk in range(5):
                    t0 = blk * ABLK
                    if blk == 0:
                        xt = xt0
                    else:
                        xt = [xa_pool.tile([128, ABLK], F32R, tag=f"xa{kc}",
                                           name=f"xa{kc}") for kc in range(KT)]
                        for kc in range(KT):
                            nc.sync.dma_start(
                                xt[kc][:],
                                x[kc * 128:(kc + 1) * 128, t0:t0 + ABLK]
                                .bitcast(F32R),
                            )
                    # x column-sums over main (non-halo) tokens for cumsum offset
                    lo = 1 - t0 if t0 < 1 else 0
                    hi = ABLK - max(0, t0 + ABLK - (TH - 1))
                    for kc in range(KT):
                        nc.vector.tensor_reduce(
                            sx_cols[:, kc, blk:blk + 1], xt[kc][:, lo:hi],
                            axis=mybir.AxisListType.X, op=OP.add,
                        )
                    for a in range(DT):
                        ps = psa.tile([128, ABLK], F32, tag="ps")
                        for kc in range(KT):
                            nc.tensor.matmul(
                                ps[:], wq[kc][:, a, :], xt[kc][:],
                                start=(kc == 0), stop=(kc == KT - 1),
                            )
                        nc.vector.tensor_scalar_add(
                            qt[:, a, t0:t0 + ABLK], ps[:], bq_sb[:, a:a + 1]
                        )

            if phases == 1:
                for a in range(DT):
                    nc.sync.dma_start(
                        r_out[a * 128:(a + 1) * 128, 0:T],
                        qt[:, a, 1:T + 1].bitcast(F32),
                    )
                return

            # ------------- phase B1: cq^T = silu(conv(Q)) -------------
            with tc.tile_pool(name="cq", bufs=1) as cq_pool:
                cq = cq_pool.tile([128, DT, T], F32R)
                with (
                    tc.tile_pool(name="wc", bufs=2) as wc_pool,
                    tc.tile_pool(name="psb", bufs=8, space="PSUM") as psb,
                ):
                    for a in range(DT):
                        wc = wc_pool.tile([128, 3, KT, 128], F32R, tag="wc")
                        for k3 in range(3):
                            nc.sync.dma_start(
                                wc[:, k3, :, :],
                                wct[k3, :, a * 128:(a + 1) * 128]
                                .rearrange("(kc p) m -> p kc m", p=128)
                                .bitcast(F32R),
                            )
                        for blk in range(T // BBLK):
                            t0 = blk * BBLK
                            ps = psb.tile([128, BBLK], F32, tag="ps")
                            first = True
                            for k3 in range(3):
                                for kc in range(KT):
                                    nc.tensor.matmul(
                                        ps[:], wc[:, k3, kc, :],
                                        qt[:, kc, t0 + k3:t0 + k3 + BBLK],
                                        start=first,
                                        stop=(k3 == 2 and kc == KT - 1),
                                    )
                                    first = False
                            sig = wc_pool.tile([128, BBLK], F32, tag="sig")
                            nc.scalar.activation(
                                sig[:], ps[:], AF.Sigmoid, bias=cb_sb[:, a:a + 1]
                            )
                            nc.vector.scalar_tensor_tensor(
                                cq[:, a, t0:t0 + BBLK], ps[:], cb_sb[:, a:a + 1],
                                sig[:], OP.add, OP.mult,
                            )

                if phases == 2:
                    for a in range(DT):
                        nc.sync.dma_start(
                            r_out[a * 128:(a + 1) * 128, 0:T],
                            cq[:, a, :].bitcast(F32),
                        )
                    return

                # ------- phase B2: E/N partial sums from exp(logits) -------
                # w_a loads first (needed immediately); W_O^T queued behind
                # it, streaming during B2 compute (offset matvec + phase C)
                with (
                    tc.tile_pool(name="wa", bufs=1) as wa_pool,
                    tc.tile_pool(name="ex", bufs=2) as ex_pool,
                    tc.tile_pool(name="psl", bufs=8, space="PSUM") as psl,
                ):
                    wa = [wa_pool.tile([128, DT, 128], F32R, tag=f"wa{kc}",
                                       name=f"wa{kc}") for kc in range(KT)]
                    for kc in range(KT):
                        nc.sync.dma_start(
                            wa[kc][:],
                            wat[kc * 128:(kc + 1) * 128, :]
                            .rearrange("p (a m) -> p a m", m=128).bitcast(F32R),
                        )
                    woo = woo_pool.tile([128, KT, DT, 128], F32R)
                    for kc in range(KT):
                        nc.sync.dma_start(
                            woo[:, kc, :, :],
                            wqt[kc * 128:(kc + 1) * 128, D:2 * D]
                            .rearrange("p (a m) -> p a m", m=128).bitcast(F32R),
                        )
                    for blk in range(T // BBLK):
                        t0 = blk * BBLK
                        for a in range(DT):
                            ps = psl.tile([128, BBLK], F32, tag="ps")
                            for kc in range(KT):
                                nc.tensor.matmul(
                                    ps[:], wa[kc][:, a, :],
                                    cq[:, kc, t0:t0 + BBLK],
                                    start=(kc == 0), stop=(kc == KT - 1),
                                )
                            expl = ex_pool.tile([128, BBLK], F32, tag="expl")
                            idx = a * 4 + blk
                            nc.scalar.activation(
                                expl[:], ps[:], AF.Exp, scale=SCALE,
                                accum_out=e_cols[:, idx:idx + 1],
                            )
                            prod = ex_pool.tile([128, BBLK], F32, tag="prod")
                            nc.vector.scalar_tensor_tensor(
                                prod[:], expl[:], 0.0,
                                qt[:, a, t0 + 1:t0 + 1 + BBLK].bitcast(F32),
                                OP.add, OP.mult,
                                accum_out=n_cols[:, idx:idx + 1],
                            )

        if phases == 3:
            nc.sync.dma_start(r_out[0:128, 0:32].rearrange("p t -> p t"), e_cols[:])
            nc.sync.dma_start(r_out[128:256, 0:32], n_cols[:])
            return

        # ------------- allreduce E, N, offv over the seq pair -------------
        nc.vector.tensor_reduce(
            stage[:, 0:DT], e_cols[:].rearrange("p (a b) -> p a b", b=4),
            axis=mybir.AxisListType.X, op=OP.add,
        )
        nc.vector.tensor_reduce(
            stage[:, DT:2 * DT], n_cols[:].rearrange("p (a b) -> p a b", b=4),
            axis=mybir.AxisListType.X, op=OP.add,
        )
        # offv = W_O @ (own x sums), masked to the first half: each core
        # computes the matvec from its LOCAL sx before the collective (PE is
        # otherwise idle here), so nothing downstream of the allreduce needs
        # the PE — the post-collective tail is pure DVE scalar work
        sx = cols.tile([128, KT], F32, name="sx")
        nc.vector.tensor_reduce(
            sx[:], sx_cols[:], axis=mybir.AxisListType.X, op=OP.add,
        )
        with tc.tile_pool(name="psv", bufs=2, space="PSUM") as psv:
            for a in range(DT):
                ps = psv.tile([128, 1], F32, tag="ps", name="ps")
                for kc in range(KT):
                    nc.tensor.matmul(
                        ps[:], woo[:, kc, a, :].bitcast(F32),
                        sx[:, kc:kc + 1],
                        start=(kc == 0), stop=(kc == KT - 1),
                    )
                nc.vector.tensor_scalar_mul(
                    stage[:, 2 * DT + a:2 * DT + a + 1], ps[:], hf0_sb[:, 0:1]
                )
        if phases == 99:
            # timing-model variant: skip the collective (TimelineSim
            # cannot model collectives); copy stage -> red locally
            nc.vector.tensor_copy(red[:], stage[:])
        else:
            cc_in = dram.tile([128, 3 * DT], F32)
            cc_out = dram.tile([128, 3 * DT], F32)
            nc.sync.dma_start(cc_in[:], stage[:])
            nc.gpsimd.collective_compute(
                "AllReduce", OP.add,
                replica_groups=[[0, 1], [2, 3], [4, 5], [6, 7]],
                ins=[cc_in.opt()], outs=[cc_out.opt()],
            )
            nc.sync.dma_start(red[:], cc_out[:])

        # ---------------- phase C: O,G -> raw cumsum -> L -> R ----------------
        # cumsum(P) = glob * (cumsum(O + b_o) + hf1*(W_O@sx + T b_o)); the
        # O/G matmuls + scan depend only on weights/x, so the collective
        # overlaps with the first `slack` blocks of that work.
        with (
            tc.tile_pool(name="wog", bufs=1) as wog_pool,
            tc.tile_pool(name="wo2", bufs=1) as wo2_pool,
            tc.tile_pool(name="xc", bufs=2) as xc_pool,
            tc.tile_pool(name="blkb", bufs=1) as blk_pool,
            tc.tile_pool(name="psc", bufs=8, space="PSUM") as psc,
        ):
            # block-0 x tiles first (the O matmuls need them right away and
            # woo is already resident), then W_G, then W_out — each needed
            # progressively later in the first C blocks
            xt_pre = xc_pool.tile([128, KT, CBLK], F32R, tag="xc", name="xt_pre")
            for kc in range(KT):
                nc.sync.dma_start(
                    xt_pre[:, kc, :],
                    x[kc * 128:(kc + 1) * 128, 1:1 + CBLK].bitcast(F32R),
                )
            wog = wog_pool.tile([128, KT, DT, 128], F32R)
            for kc in range(KT):
                nc.sync.dma_start(
                    wog[:, kc, :, :],
                    wqt[kc * 128:(kc + 1) * 128, 2 * D:3 * D]
                    .rearrange("p (a m) -> p a m", m=128).bitcast(F32R),
                )
            wo2 = wo2_pool.tile([128, KT, DT, 128], F32R)
            for kc in range(KT):
                nc.sync.dma_start(
                    wo2[:, kc, :, :],
                    wot[kc * 128:(kc + 1) * 128, :]
                    .rearrange("p (a m) -> p a m", m=128).bitcast(F32R),
                )

            # hf1 * T * b_o, needed by the offset tail (no allreduce dep)
            bo_th = cols.tile([128, DT], F32)
            nc.vector.tensor_scalar(
                bo_th[:], bo_sb[:], float(T), hf1_sb[:, 0:1], OP.mult, OP.mult,
            )

            nblk = T // CBLK
            if 50 <= phases < 99:
                nblk = phases - 50
            slack = min(SLACK, max(nblk - 1, 0))

            def c_og_part(blk, c_prev, xt=None):
                t0 = blk * CBLK
                if xt is None:
                    xt = xc_pool.tile([128, KT, CBLK], F32R, tag="xc", name="xt")
                    for kc in range(KT):
                        nc.sync.dma_start(
                            xt[:, kc, :],
                            x[kc * 128:(kc + 1) * 128, t0 + 1:t0 + 1 + CBLK]
                            .bitcast(F32R),
                        )
                pt = blk_pool.tile([128, DT, CBLK], F32, tag="pt", bufs=2,
                                   name="pt")
                ct = blk_pool.tile([128, DT, CBLK], F32, tag="ct",
                                   bufs=slack + 1, name="ct")
                gt = blk_pool.tile([128, DT, CBLK], F32, tag="gt",
                                   bufs=slack + 1, name="gt")
                carry = xc_pool.tile([128, DT], F32, tag="carry", name="carry")
                for a in range(DT):
                    ps = psc.tile([128, CBLK], F32, tag="ps", name="ps")
                    for kc in range(KT):
                        nc.tensor.matmul(
                            ps[:], woo[:, kc, a, :], xt[:, kc, :],
                            start=(kc == 0), stop=(kc == KT - 1),
                        )
                    # pt = O + b_o (glob applied later, after the allreduce)
                    nc.scalar.activation(
                        pt[:, a, :], ps[:], AF.Identity, bias=bo_sb[:, a:a + 1]
                    )
                    init = (zcol[:, 0:1] if c_prev is None
                            else c_prev[:, a:a + 1])
                    nc.vector.tensor_tensor_scan(
                        ct[:, a, :], pt[:, a, :], pt[:, a, :], init,
                        OP.add, OP.bypass,
                    )
                # carry the last cumsum column via ACT so the next
                # block's scan does not read a scan output directly
                nc.scalar.copy(carry[:], ct[:, :, CBLK - 1:CBLK])
                for a in range(DT):
                    ps = psc.tile([128, CBLK], F32, tag="ps", name="ps")
                    for kc in range(KT):
                        nc.tensor.matmul(
                            ps[:], wog[:, kc, a, :], xt[:, kc, :],
                            start=(kc == 0), stop=(kc == KT - 1),
                        )
                    sig = xc_pool.tile([128, CBLK], F32, tag="sig", name="sig")
                    nc.scalar.activation(
                        sig[:], ps[:], AF.Sigmoid, bias=bg_sb[:, a:a + 1]
                    )
                    nc.vector.scalar_tensor_tensor(
                        gt[:, a, :], ps[:], bg_sb[:, a:a + 1], sig[:],
                        OP.add, OP.mult,
                    )
                return ct, gt, carry

            def allreduce_tail():
                # everything downstream of `red`; emitted after `slack`
                # blocks of O/G work so the engine queues stay busy while
                # the collective completes
                recip = cols.tile([128, DT], F32, name="recip")
                nc.vector.reciprocal(recip[:], red[:, 0:DT])
                nc.vector.tensor_mul(glob[:], red[:, DT:2 * DT], recip[:])
                # R = (gt*(ct0+offh)) @ (W_out^T scaled by glob along the
                # contraction): fold glob into wo2's columns once instead of
                # scaling every C block (glob is constant over tokens).
                # DVE only: the f32r-consumed output needs DVE's f32r rounding.
                for kc in range(KT):
                    nc.vector.tensor_scalar_mul(
                        wo2[:, kc, :, :],
                        wo2[:, kc, :, :].bitcast(F32), glob[:, kc:kc + 1],
                    )
                # offh = hf1*offv + hf1*T*b_o; offv arrived via the allreduce
                nc.vector.scalar_tensor_tensor(
                    offh[:], red[:, 2 * DT:3 * DT], hf1_sb[:, 0:1], bo_th[:],
                    OP.mult, OP.add,
                )

            def c_w_part(blk, ct, gt):
                t0 = blk * CBLK
                lt = blk_pool.tile([128, DT, CBLK], F32R, tag="lt", bufs=2,
                                   name="lt")
                rt = blk_pool.tile([128, DT, CBLK], F32, tag="rt", bufs=1,
                                   name="rt")
                for a in range(DT):
                    # lt = gt * (raw cumsum + offset column); glob lives in wo2
                    nc.vector.scalar_tensor_tensor(
                        lt[:, a, :], ct[:, a, :], offh[:, a:a + 1],
                        gt[:, a, :], OP.add, OP.mult,
                    )
                for a in range(DT):
                    ps = psc.tile([128, CBLK], F32, tag="ps", name="ps")
                    for kc in range(KT):
                        nc.tensor.matmul(
                            ps[:], wo2[:, kc, a, :], lt[:, kc, :],
                            start=(kc == 0), stop=(kc == KT - 1),
                        )
                    nc.scalar.activation(
                        rt[:, a, :], ps[:], AF.Identity,
                        bias=bout_sb[:, a:a + 1],
                    )
                    nc.sync.dma_start(
                        r_out[a * 128:(a + 1) * 128, t0:t0 + CBLK],
                        rt[:, a, :],
                    )

            if phases == 4:
                allreduce_tail()
                nc.sync.dma_start(r_out[0:128, 0:DT], offh[:])
                nc.sync.dma_start(r_out[128:256, 0:DT], glob[:])
                return

            pending = []
            c_prev = None
            tail_done = False
            for blk in range(nblk):
                ct, gt, carry = c_og_part(blk, c_prev,
                                          xt=xt_pre if blk == 0 else None)
                c_prev = carry
                pending.append((blk, ct, gt))
                if blk >= slack:
                    if not tail_done:
                        allreduce_tail()
                        tail_done = True
                    b2, ct2, gt2 = pending.pop(0)
                    c_w_part(b2, ct2, gt2)
            if not tail_done:
                allreduce_tail()
            for b2, ct2, gt2 in pending:
                c_w_part(b2, ct2, gt2)


_CACHE = {}


def _build(phases=5):
    if phases in _CACHE:
        return _CACHE[phases]
    nc = bacc.Bacc(None, target_bir_lowering=False, num_devices=N_CORES)
    prm = {
        "x": nc.declare_dram_parameter("x", [DM, TH], F32, isOutput=False),
        "wqt": nc.declare_dram_parameter("wqt", [DM, 3 * D], F32, isOutput=False),
        "wct": nc.declare_dram_parameter("wct", [3, D, D], F32, isOutput=False),
        "wat": nc.declare_dram_parameter("wat", [D, D], F32, isOutput=False),
        "wot": nc.declare_dram_parameter("wot", [D, D], F32, isOutput=False),
        "bq": nc.declare_dram_parameter("bq", [128, DT], F32, isOutput=False),
        "bo": nc.declare_dram_parameter("bo", [128, DT], F32, isOutput=False),
        "bg": nc.declare_dram_parameter("bg", [128, DT], F32, isOutput=False),
        "cb": nc.declare_dram_parameter("cb", [128, DT], F32, isOutput=False),
        "bout": nc.declare_dram_parameter("bout", [128, DT], F32, isOutput=False),
        "hf0": nc.declare_dram_parameter("hf0", [128, 1], F32, isOutput=False),
        "hf1": nc.declare_dram_parameter("hf1", [128, 1], F32, isOutput=False),
        "r": nc.declare_dram_parameter("r", [DM, T], F32, isOutput=True),
    }
    with tile.TileContext(nc, num_cores=N_CORES) as tc:
        _emit(tc, nc, prm, phases)
    nc.compile()
    _CACHE[phases] = nc
    return nc


def make_in_maps(x, W_qog, b_qog, conv_w, conv_b, w_a, W_out, b_out):
    f = np.float32
    x = np.asarray(x, f)
    wqt = np.ascontiguousarray(np.asarray(W_qog, f).T)          # [dm, 3d]
    wct = np.ascontiguousarray(np.asarray(conv_w, f).transpose(2, 1, 0))
    wat = np.ascontiguousarray(np.asarray(w_a, f).T)
    wot = np.ascontiguousarray(np.asarray(W_out, f).T)

    def col(v):  # [d] -> [128, DT] with d = a*128 + p
        return np.ascontiguousarray(np.asarray(v, f).reshape(DT, 128).T)

    b_qog = np.asarray(b_qog, f)
    bq, bo, bg = col(b_qog[:D]), col(b_qog[D:2 * D]), col(b_qog[2 * D:])
    cb, bout = col(conv_b), col(b_out)

    in_maps = []
    for c in range(N_CORES):
        b, h = c // 2, c % 2
        t0 = h * T
        xs = np.zeros((TH, DM), f)
        xs[1:T + 1] = x[b, t0:t0 + T]
        if t0 > 0:
            xs[0] = x[b, t0 - 1]
        if t0 + T < S:
            xs[T + 1] = x[b, t0 + T]
        xs = np.ascontiguousarray(xs.T)            # [DM, TH] feature-major
        in_maps.append({
            "x": xs, "wqt": wqt, "wct": wct, "wat": wat, "wot": wot,
            "bq": bq, "bo": bo, "bg": bg, "cb": cb, "bout": bout,
            "hf0": np.full((128, 1), 1.0 - h, f),
            "hf1": np.full((128, 1), float(h), f),
        })
    return in_maps


def kernel(x, W_qog, b_qog, conv_w, conv_b, w_a, W_out, b_out):
    nc = _build(5)
    in_maps = make_in_maps(x, W_qog, b_qog, conv_w, conv_b, w_a, W_out, b_out)
    res = None
    for attempt in range(5):
        try:
            res = run_bass_kernel_spmd(nc, in_maps, list(range(N_CORES)))
            break
        except Exception:
            # the execution path through the device bridge is occasionally
            # flaky (worker hangup / mesh desync); reset and retry
            if attempt == 4:
                raise
            import jax

            try:
                jax.clear_backends()
            except Exception:
                pass
            import time

            time.sleep(5 + 5 * attempt)
    out = np.empty((B, S, DM), np.float32)
    for c in range(N_CORES):
        b, h = c // 2, c % 2
        out[b, h * T:(h + 1) * T, :] = res.results[c]["r"].T
    return out


# revision 5
# speedup vs baseline: 16.2632x; 1.0964x over previous
"""F2NetHead Trainium2 kernel (8 NeuronCores, Bass/Tile) — v2.

Reference computation (per batch b):
    qog = x @ W_qog.T + b_qog ; Q,O,G = split(qog)
    cq  = silu(conv1d(Q, conv_w, pad=1) + conv_b)          # mixes channels
    l   = (cq @ w_a.T) / sqrt(d)
    attn= softmax(l, axis=seq)
    glob= sum_seq(Q * attn)                                 # [1, d]
    P   = O * glob
    L   = silu(G) * cumsum(P, axis=seq)
    R   = L @ W_out.T + b_out

Sharding: 8 cores = 4 batches x 2 sequence halves; x arrives with a
1-token halo so the conv needs no neighbor exchange. Per batch pair the
only comm is an AllReduce of 3 [d] vectors: E = sum exp(l),
N = sum Q*exp(l), sx = sum of half-0's x rows.

v2 changes vs v1:
  *  glob factors out of the cumsum:  cumsum(P) = glob * cumsum(O + b_o)
     and the half-1 offset = glob * (W_O @ sx + T b_o).  Phase C's O/G
     matmuls and the raw cumsum therefore have NO dependency on the
     collective; only a per-block tensor_scalar (+offset, *glob) and the
     W_out matmul consume the allreduce result.  SLACK blocks of O/G
     work are emitted before the first allreduce-dependent instruction
     on each engine queue, hiding the collective latency.
  *  weight DMA is staged: W_Q before phase A, conv weights + w_a during
     B1, W_O during B2, W_G/W_out at C start — removes the 8 MiB
     startup stall and the B1->B2 weight-load gap.

On-chip layout is feature-major ([d partitions, tokens free]) so every
sequence-axis op (softmax sums, global sum, cumsum) is a free-dim op.
All matmuls run in float32r (full PE rate); silu is x*sigmoid(x).
"""

import numpy as np

import concourse.bacc as bacc
import concourse.mybir as mybir
import concourse.tile as tile
from concourse.bass_utils import run_bass_kernel_spmd

F32 = mybir.dt.float32
F32R = mybir.dt.float32r
AF = mybir.ActivationFunctionType
OP = mybir.AluOpType

B, S, D, DM = 4, 4096, 1024, 1024
N_CORES = 8
T = S // 2            # tokens per core
TH = T + 2            # with halo
DT = D // 128         # d tiles (8)
KT = DM // 128        # contraction tiles (8)
ABLK = 410            # phase A token block (5 blocks over TH=2050)
BBLK = 512            # phase B token block (4 blocks over T)
CBLK = 256            # phase C token block (8 blocks over T)
SLACK = 2             # C blocks of O/G work emitted ahead of any
                      # allreduce-dependent instruction
SCALE = 1.0 / float(np.sqrt(D))


def _emit(tc, nc, prm, phases=5):
    reps = 1
    if phases >= 100:
        reps, phases = phases // 100, 5
    for _ in range(reps):
        _emit_once(tc, nc, prm, phases)


def _emit_once(tc, nc, prm, phases):
    x, w_all, allb_d, r_out = prm["x"], prm["w"], prm["allb"], prm["r"]
    # packed weight buffer column offsets: [wqt | wat | wot | wct k3=0,1,2]
    wqt = w_all
    C_WA, C_WO, C_WC = 3 * D, 4 * D, 5 * D

    with (
        tc.tile_pool(name="cols", bufs=1) as cols,
        tc.tile_pool(name="woo", bufs=1) as woo_pool,
        tc.tile_pool(name="dram", bufs=1, space="DRAM") as dram,
    ):
        # per-partition bias / flag columns, packed [128, 5*DT+2]
        # ([bq|bo|bg|cb|bout|hf0|hf1], d = a*128 + p within each chunk)
        allb = cols.tile([128, 5 * DT + 2], F32)
        nc.sync.dma_start(allb[:], allb_d[:])
        bq_sb = allb[:, 0:DT]
        bo_sb = allb[:, DT:2 * DT]
        bg_sb = allb[:, 2 * DT:3 * DT]
        cb_sb = allb[:, 3 * DT:4 * DT]
        bout_sb = allb[:, 4 * DT:5 * DT]
        hf0_sb = allb[:, 5 * DT:5 * DT + 1]
        hf1_sb = allb[:, 5 * DT + 1:5 * DT + 2]

        # accumulators that survive across phases
        sx_cols = cols.tile([128, KT, 5], F32)      # per-A-block x sums
        e_cols = cols.tile([128, DT * 4], F32)      # per-(a,B-block) exp sums
        n_cols = cols.tile([128, DT * 4], F32)      # per-(a,B-block) Q*exp sums
        stage = cols.tile([128, 3 * DT], F32)       # allreduce staging
        red = cols.tile([128, 3 * DT], F32)         # allreduce result
        glob = cols.tile([128, DT], F32)
        offh = cols.tile([128, DT], F32)            # hf1*(W_O@sx + T b_o)
        zcol = cols.tile([128, 1], F32)             # scan init for block 0
        nc.vector.memset(zcol[:], 0.0)

        # ---------------- phase A: Q^T over TH halo'd tokens ----------------
        with tc.tile_pool(name="qt", bufs=1) as qt_pool:
            qt = qt_pool.tile([128, DT, TH], F32R)
            with (
                tc.tile_pool(name="wq", bufs=1) as wq_pool,
                tc.tile_pool(name="xa", bufs=2) as xa_pool,
                tc.tile_pool(name="psa", bufs=8, space="PSUM") as psa,
            ):
                wq = [wq_pool.tile([128, DT, 128], F32R, tag=f"wq{kc}",
                                   name=f"wq{kc}") for kc in range(KT)]
                # block-0 x tiles interleave with the wq loads so the first
                # contraction chain can start as soon as pair 0 lands
                xt0 = [xa_pool.tile([128, ABLK], F32R, tag=f"xa{kc}",
                                    name=f"xa{kc}") for kc in range(KT)]
                for kc in range(KT):
                    nc.sync.dma_start(
                        wq[kc][:],
                        wqt[kc * 128:(kc + 1) * 128, 0:D]
                        .rearrange("p (a m) -> p a m", m=128).bitcast(F32R),
                    )
                    nc.sync.dma_start(
                        xt0[kc][:],
                        x[kc * 128:(kc + 1) * 128, 0:ABLK].bitcast(F32R),
                    )
                for blk in range(5):
                    t0 = blk * ABLK
                    if blk == 0:
                        xt = xt0
                    else:
                        xt = [xa_pool.tile([128, ABLK], F32R, tag=f"xa{kc}",
                                           name=f"xa{kc}") for kc in range(KT)]
                        for kc in range(KT):
                            nc.sync.dma_start(
                                xt[kc][:],
                                x[kc * 128:(kc + 1) * 128, t0:t0 + ABLK]
                                .bitcast(F32R),
                            )
                    # x column-sums over main (non-halo) tokens for cumsum offset
                    lo = 1 - t0 if t0 < 1 else 0
                    hi = ABLK - max(0, t0 + ABLK - (TH - 1))
                    for kc in range(KT):
                        nc.vector.tensor_reduce(
                            sx_cols[:, kc, blk:blk + 1], xt[kc][:, lo:hi],
                            axis=mybir.AxisListType.X, op=OP.add,
                        )
                    for a in range(DT):
                        ps = psa.tile([128, ABLK], F32, tag="ps")
                        for kc in range(KT):
                            nc.tensor.matmul(
                                ps[:], wq[kc][:, a, :], xt[kc][:],
                                start=(kc == 0), stop=(kc == KT - 1),
                            )
                        nc.vector.tensor_scalar_add(
                            qt[:, a, t0:t0 + ABLK], ps[:], bq_sb[:, a:a + 1]
                        )

            if phases == 1:
                for a in range(DT):
                    nc.sync.dma_start(
                        r_out[a * 128:(a + 1) * 128, 0:T],
                        qt[:, a, 1:T + 1].bitcast(F32),
                    )
                return

            # ------------- phase B1: cq^T = silu(conv(Q)) -------------
            with tc.tile_pool(name="cq", bufs=1) as cq_pool:
                cq = cq_pool.tile([128, DT, T], F32R)
                with (
                    tc.tile_pool(name="wc", bufs=2) as wc_pool,
                    tc.tile_pool(name="psb", bufs=8, space="PSUM") as psb,
                ):
                    for a in range(DT):
                        wc = wc_pool.tile([128, 3, KT, 128], F32R, tag="wc")
                        for k3 in range(3):
                            c0 = C_WC + k3 * D + a * 128
                            nc.sync.dma_start(
                                wc[:, k3, :, :],
                                w_all[:, c0:c0 + 128]
                                .rearrange("(kc p) m -> p kc m", p=128)
                                .bitcast(F32R),
                            )
                        for blk in range(T // BBLK):
                            t0 = blk * BBLK
                            ps = psb.tile([128, BBLK], F32, tag="ps")
                            first = True
                            for k3 in range(3):
                                for kc in range(KT):
                                    nc.tensor.matmul(
                                        ps[:], wc[:, k3, kc, :],
                                        qt[:, kc, t0 + k3:t0 + k3 + BBLK],
                                        start=first,
                                        stop=(k3 == 2 and kc == KT - 1),
                                    )
                                    first = False
                            sig = wc_pool.tile([128, BBLK], F32, tag="sig")
                            nc.scalar.activation(
                                sig[:], ps[:], AF.Sigmoid, bias=cb_sb[:, a:a + 1]
                            )
                            nc.vector.scalar_tensor_tensor(
                                cq[:, a, t0:t0 + BBLK], ps[:], cb_sb[:, a:a + 1],
                                sig[:], OP.add, OP.mult,
                            )

                if phases == 2:
                    for a in range(DT):
                        nc.sync.dma_start(
                            r_out[a * 128:(a + 1) * 128, 0:T],
                            cq[:, a, :].bitcast(F32),
                        )
                    return

                # ------- phase B2: E/N partial sums from exp(logits) -------
                # w_a loads first (needed immediately); W_O^T queued behind
                # it, streaming during B2 compute (offset matvec + phase C)
                with (
                    tc.tile_pool(name="wa", bufs=1) as wa_pool,
                    tc.tile_pool(name="ex", bufs=2) as ex_pool,
                    tc.tile_pool(name="psl", bufs=8, space="PSUM") as psl,
                ):
                    wa = [wa_pool.tile([128, DT, 128], F32R, tag=f"wa{kc}",
                                       name=f"wa{kc}") for kc in range(KT)]
                    for kc in range(KT):
                        nc.sync.dma_start(
                            wa[kc][:],
                            w_all[kc * 128:(kc + 1) * 128, C_WA:C_WA + D]
                            .rearrange("p (a m) -> p a m", m=128).bitcast(F32R),
                        )
                    woo = woo_pool.tile([128, KT, DT, 128], F32R)
                    for kc in range(KT):
                        nc.sync.dma_start(
                            woo[:, kc, :, :],
                            wqt[kc * 128:(kc + 1) * 128, D:2 * D]
                            .rearrange("p (a m) -> p a m", m=128).bitcast(F32R),
                        )
                    for blk in range(T // BBLK):
                        t0 = blk * BBLK
                        for a in range(DT):
                            ps = psl.tile([128, BBLK], F32, tag="ps")
                            for kc in range(KT):
                                nc.tensor.matmul(
                                    ps[:], wa[kc][:, a, :],
                                    cq[:, kc, t0:t0 + BBLK],
                                    start=(kc == 0), stop=(kc == KT - 1),
                                )
                            expl = ex_pool.tile([128, BBLK], F32, tag="expl")
                            idx = a * 4 + blk
                            nc.scalar.activation(
                                expl[:], ps[:], AF.Exp, scale=SCALE,
                                accum_out=e_cols[:, idx:idx + 1],
                            )
                            prod = ex_pool.tile([128, BBLK], F32, tag="prod")
                            nc.vector.scalar_tensor_tensor(
                                prod[:], expl[:], 0.0,
                                qt[:, a, t0 + 1:t0 + 1 + BBLK].bitcast(F32),
                                OP.add, OP.mult,
                                accum_out=n_cols[:, idx:idx + 1],
                            )

        if phases == 3:
            nc.sync.dma_start(r_out[0:128, 0:32].rearrange("p t -> p t"), e_cols[:])
            nc.sync.dma_start(r_out[128:256, 0:32], n_cols[:])
            return

        # ------------- allreduce E, N, offv over the seq pair -------------
        nc.vector.tensor_reduce(
            stage[:, 0:DT], e_cols[:].rearrange("p (a b) -> p a b", b=4),
            axis=mybir.AxisListType.X, op=OP.add,
        )
        nc.vector.tensor_reduce(
            stage[:, DT:2 * DT], n_cols[:].rearrange("p (a b) -> p a b", b=4),
            axis=mybir.AxisListType.X, op=OP.add,
        )
        # offv = W_O @ (own x sums), masked to the first half: each core
        # computes the matvec from its LOCAL sx before the collective (PE is
        # otherwise idle here), so nothing downstream of the allreduce needs
        # the PE — the post-collective tail is pure DVE scalar work
        sx = cols.tile([128, KT], F32, name="sx")
        nc.vector.tensor_reduce(
            sx[:], sx_cols[:], axis=mybir.AxisListType.X, op=OP.add,
        )
        with tc.tile_pool(name="psv", bufs=2, space="PSUM") as psv:
            for a in range(DT):
                ps = psv.tile([128, 1], F32, tag="ps", name="ps")
                for kc in range(KT):
                    nc.tensor.matmul(
                        ps[:], woo[:, kc, a, :].bitcast(F32),
                        sx[:, kc:kc + 1],
                        start=(kc == 0), stop=(kc == KT - 1),
                    )
                nc.vector.tensor_scalar_mul(
                    stage[:, 2 * DT + a:2 * DT + a + 1], ps[:], hf0_sb[:, 0:1]
                )
        if phases == 99:
            # timing-model variant: skip the collective (TimelineSim
            # cannot model collectives); copy stage -> red locally
            nc.vector.tensor_copy(red[:], stage[:])
        else:
            cc_in = dram.tile([128, 3 * DT], F32)
            cc_out = dram.tile([128, 3 * DT], F32)
            nc.sync.dma_start(cc_in[:], stage[:])
            nc.gpsimd.collective_compute(
                "AllReduce", OP.add,
                replica_groups=[[0, 1], [2, 3], [4, 5], [6, 7]],
                ins=[cc_in.opt()], outs=[cc_out.opt()],
            )
            nc.sync.dma_start(red[:], cc_out[:])

        # ---------------- phase C: O,G -> raw cumsum -> L -> R ----------------
        # cumsum(P) = glob * (cumsum(O + b_o) + hf1*(W_O@sx + T b_o)); the
        # O/G matmuls + scan depend only on weights/x, so the collective
        # overlaps with the first `slack` blocks of that work.
        with (
            tc.tile_pool(name="wog", bufs=1) as wog_pool,
            tc.tile_pool(name="wo2", bufs=1) as wo2_pool,
            tc.tile_pool(name="xc", bufs=2) as xc_pool,
            tc.tile_pool(name="blkb", bufs=1) as blk_pool,
            tc.tile_pool(name="psc", bufs=8, space="PSUM") as psc,
        ):
            # block-0 x tiles first (the O matmuls need them right away and
            # woo is already resident), then W_G, then W_out — each needed
            # progressively later in the first C blocks
            xt_pre = xc_pool.tile([128, KT, CBLK], F32R, tag="xc", name="xt_pre")
            for kc in range(KT):
                nc.sync.dma_start(
                    xt_pre[:, kc, :],
                    x[kc * 128:(kc + 1) * 128, 1:1 + CBLK].bitcast(F32R),
                )
            wog = wog_pool.tile([128, KT, DT, 128], F32R)
            for kc in range(KT):
                nc.sync.dma_start(
                    wog[:, kc, :, :],
                    wqt[kc * 128:(kc + 1) * 128, 2 * D:3 * D]
                    .rearrange("p (a m) -> p a m", m=128).bitcast(F32R),
                )
            wo2 = wo2_pool.tile([128, KT, DT, 128], F32R)
            for kc in range(KT):
                nc.sync.dma_start(
                    wo2[:, kc, :, :],
                    w_all[kc * 128:(kc + 1) * 128, C_WO:C_WO + D]
                    .rearrange("p (a m) -> p a m", m=128).bitcast(F32R),
                )

            # hf1 * T * b_o, needed by the offset tail (no allreduce dep)
            bo_th = cols.tile([128, DT], F32)
            nc.vector.tensor_scalar(
                bo_th[:], bo_sb[:], float(T), hf1_sb[:, 0:1], OP.mult, OP.mult,
            )

            nblk = T // CBLK
            if 50 <= phases < 99:
                nblk = phases - 50
            slack = min(SLACK, max(nblk - 1, 0))

            def c_og_part(blk, c_prev, xt=None):
                t0 = blk * CBLK
                if xt is None:
                    xt = xc_pool.tile([128, KT, CBLK], F32R, tag="xc", name="xt")
                    for kc in range(KT):
                        nc.sync.dma_start(
                            xt[:, kc, :],
                            x[kc * 128:(kc + 1) * 128, t0 + 1:t0 + 1 + CBLK]
                            .bitcast(F32R),
                        )
                pt = blk_pool.tile([128, DT, CBLK], F32, tag="pt", bufs=2,
                                   name="pt")
                ct = blk_pool.tile([128, DT, CBLK], F32, tag="ct",
                                   bufs=slack + 1, name="ct")
                gt = blk_pool.tile([128, DT, CBLK], F32, tag="gt",
                                   bufs=slack + 1, name="gt")
                carry = xc_pool.tile([128, DT], F32, tag="carry", name="carry")
                for a in range(DT):
                    ps = psc.tile([128, CBLK], F32, tag="ps", name="ps")
                    for kc in range(KT):
                        nc.tensor.matmul(
                            ps[:], woo[:, kc, a, :], xt[:, kc, :],
                            start=(kc == 0), stop=(kc == KT - 1),
                        )
                    # pt = O + b_o (glob applied later, after the allreduce)
                    nc.scalar.activation(
                        pt[:, a, :], ps[:], AF.Identity, bias=bo_sb[:, a:a + 1]
                    )
                    init = (zcol[:, 0:1] if c_prev is None
                            else c_prev[:, a:a + 1])
                    nc.vector.tensor_tensor_scan(
                        ct[:, a, :], pt[:, a, :], pt[:, a, :], init,
                        OP.add, OP.bypass,
                    )
                # carry the last cumsum column via ACT so the next
                # block's scan does not read a scan output directly
                nc.scalar.copy(carry[:], ct[:, :, CBLK - 1:CBLK])
                for a in range(DT):
                    ps = psc.tile([128, CBLK], F32, tag="ps", name="ps")
                    for kc in range(KT):
                        nc.tensor.matmul(
                            ps[:], wog[:, kc, a, :], xt[:, kc, :],
                            start=(kc == 0), stop=(kc == KT - 1),
                        )
                    sig = xc_pool.tile([128, CBLK], F32, tag="sig", name="sig")
                    nc.scalar.activation(
                        sig[:], ps[:], AF.Sigmoid, bias=bg_sb[:, a:a + 1]
                    )
                    nc.vector.scalar_tensor_tensor(
                        gt[:, a, :], ps[:], bg_sb[:, a:a + 1], sig[:],
                        OP.add, OP.mult,
                    )
                return ct, gt, carry

            def allreduce_tail():
                # everything downstream of `red`; emitted after `slack`
                # blocks of O/G work so the engine queues stay busy while
                # the collective completes
                recip = cols.tile([128, DT], F32, name="recip")
                nc.vector.reciprocal(recip[:], red[:, 0:DT])
                nc.vector.tensor_mul(glob[:], red[:, DT:2 * DT], recip[:])
                # R = (gt*(ct0+offh)) @ (W_out^T scaled by glob along the
                # contraction): fold glob into wo2's columns once instead of
                # scaling every C block (glob is constant over tokens).
                # DVE only: the f32r-consumed output needs DVE's f32r rounding.
                for kc in range(KT):
                    nc.vector.tensor_scalar_mul(
                        wo2[:, kc, :, :],
                        wo2[:, kc, :, :].bitcast(F32), glob[:, kc:kc + 1],
                    )
                # offh = hf1*offv + hf1*T*b_o; offv arrived via the allreduce
                nc.vector.scalar_tensor_tensor(
                    offh[:], red[:, 2 * DT:3 * DT], hf1_sb[:, 0:1], bo_th[:],
                    OP.mult, OP.add,
                )

            def c_w_part(blk, ct, gt):
                t0 = blk * CBLK
                lt = blk_pool.tile([128, DT, CBLK], F32R, tag="lt", bufs=2,
                                   name="lt")
                rt = blk_pool.tile([128, DT, CBLK], F32, tag="rt", bufs=1,
                                   name="rt")
                for a in range(DT):
                    # lt = gt * (raw cumsum + offset column); glob lives in wo2
                    nc.vector.scalar_tensor_tensor(
                        lt[:, a, :], ct[:, a, :], offh[:, a:a + 1],
                        gt[:, a, :], OP.add, OP.mult,
                    )
                for a in range(DT):
                    ps = psc.tile([128, CBLK], F32, tag="ps", name="ps")
                    for kc in range(KT):
                        nc.tensor.matmul(
                            ps[:], wo2[:, kc, a, :], lt[:, kc, :],
                            start=(kc == 0), stop=(kc == KT - 1),
                        )
                    nc.scalar.activation(
                        rt[:, a, :], ps[:], AF.Identity,
                        bias=bout_sb[:, a:a + 1],
                    )
                    nc.sync.dma_start(
                        r_out[a * 128:(a + 1) * 128, t0:t0 + CBLK],
                        rt[:, a, :],
                    )

            if phases == 4:
                allreduce_tail()
                nc.sync.dma_start(r_out[0:128, 0:DT], offh[:])
                nc.sync.dma_start(r_out[128:256, 0:DT], glob[:])
                return

            pending = []
            c_prev = None
            tail_done = False
            for blk in range(nblk):
                ct, gt, carry = c_og_part(blk, c_prev,
                                          xt=xt_pre if blk == 0 else None)
                c_prev = carry
                pending.append((blk, ct, gt))
                if blk >= slack:
                    if not tail_done:
                        allreduce_tail()
                        tail_done = True
                    b2, ct2, gt2 = pending.pop(0)
                    c_w_part(b2, ct2, gt2)
            if not tail_done:
                allreduce_tail()
            for b2, ct2, gt2 in pending:
                c_w_part(b2, ct2, gt2)


_CACHE = {}


def _build(phases=5):
    if phases in _CACHE:
        return _CACHE[phases]
    nc = bacc.Bacc(None, target_bir_lowering=False, num_devices=N_CORES)
    prm = {
        "x": nc.declare_dram_parameter("x", [DM, TH], F32, isOutput=False),
        "w": nc.declare_dram_parameter("w", [DM, 8 * D], F32, isOutput=False),
        "allb": nc.declare_dram_parameter("allb", [128, 5 * DT + 2], F32,
                                          isOutput=False),
        "r": nc.declare_dram_parameter("r", [DM, T], F32, isOutput=True),
    }
    with tile.TileContext(nc, num_cores=N_CORES) as tc:
        _emit(tc, nc, prm, phases)
    nc.compile()
    _CACHE[phases] = nc
    return nc


def make_in_maps(x, W_qog, b_qog, conv_w, conv_b, w_a, W_out, b_out):
    f = np.float32
    x = np.asarray(x, f)
    wqt = np.ascontiguousarray(np.asarray(W_qog, f).T)          # [dm, 3d]
    wct = np.ascontiguousarray(np.asarray(conv_w, f).transpose(2, 1, 0))
    wat = np.ascontiguousarray(np.asarray(w_a, f).T)
    wot = np.ascontiguousarray(np.asarray(W_out, f).T)

    def col(v):  # [d] -> [128, DT] with d = a*128 + p
        return np.ascontiguousarray(np.asarray(v, f).reshape(DT, 128).T)

    b_qog = np.asarray(b_qog, f)
    bq, bo, bg = col(b_qog[:D]), col(b_qog[D:2 * D]), col(b_qog[2 * D:])
    cb, bout = col(conv_b), col(b_out)
    # one packed weight buffer: [wqt | wat | wot | wct k3=0,1,2]  [DM, 8D]
    w_all = np.ascontiguousarray(
        np.concatenate([wqt, wat, wot, wct[0], wct[1], wct[2]], axis=1)
    )

    in_maps = []
    for c in range(N_CORES):
        b, h = c // 2, c % 2
        t0 = h * T
        xs = np.zeros((TH, DM), f)
        xs[1:T + 1] = x[b, t0:t0 + T]
        if t0 > 0:
            xs[0] = x[b, t0 - 1]
        if t0 + T < S:
            xs[T + 1] = x[b, t0 + T]
        xs = np.ascontiguousarray(xs.T)            # [DM, TH] feature-major
        allb = np.concatenate(
            [bq, bo, bg, cb, bout,
             np.full((128, 1), 1.0 - h, f), np.full((128, 1), float(h), f)],
            axis=1,
        )
        in_maps.append({"x": xs, "w": w_all, "allb": np.ascontiguousarray(allb)})
    return in_maps


def kernel(x, W_qog, b_qog, conv_w, conv_b, w_a, W_out, b_out):
    nc = _build(5)
    in_maps = make_in_maps(x, W_qog, b_qog, conv_w, conv_b, w_a, W_out, b_out)
    res = None
    for attempt in range(5):
        try:
            res = run_bass_kernel_spmd(nc, in_maps, list(range(N_CORES)))
            break
        except Exception:
            # the execution path through the device bridge is occasionally
            # flaky (worker hangup / mesh desync); reset and retry
            if attempt == 4:
                raise
            import jax

            try:
                jax.clear_backends()
            except Exception:
                pass
            import time

            time.sleep(5 + 5 * attempt)
    out = np.empty((B, S, DM), np.float32)
    for c in range(N_CORES):
        b, h = c // 2, c % 2
        out[b, h * T:(h + 1) * T, :] = res.results[c]["r"].T
    return out
